# revision 23
# baseline (speedup 1.0000x reference)
"""Multi-head causal self-attention on 8 TRN2 NeuronCores (bf16 + fp8 QK).

Problem (nn_MultiHeadAttention): B=2, T=2048, C=1024, H=16 heads, hs=64.
  q,k,v = per-head projections of x; causal softmax(q k^T / 8) v;
  concat heads; out = att @ Wo + bo.

Sharding: core c in 0..7 -> (batch b = c//4, head-group g = c%4, 4 heads each).
Each core computes Q/K/V + causal attention for its 4 heads on its batch,
normalized attention outputs are AllGathered across the 4 cores of the same
batch (replica groups [0-3], [4-7]), then each core computes a disjoint
256-column slice of the output projection (column-parallel Wo) + bias slice.
Host does a pure concat of the 8 disjoint output slices.

Q/K projections run in fp8e4 DoubleRow mode: W (x64) and x (x4) are quantized
host-side, the PE contracts TWO 128-row c-chunks per pass at 0.5 cycles/row
(4x the bf16 rate), and the combined 2^16 scale is folded into the softmax
exp scale (0.125 / 65536).  Quantization error enters only through the
softmax logits, where it attenuates to ~1% of the output.  V, the attention
matmuls, and the output projection stay bf16 (their error hits the output
directly); PSUM accumulation is fp32 throughout.

Attention layout: scores stay transposed (S^T [s, t], lhsT=K^T, rhs=Q^T),
exp runs on ACT into bf16 P^T tiles, and AV is computed non-transposed:
out[t_p, d_f] += P^T[s, t-tile].T @ V[s, d] with full 128 output partitions.
Softmax denominators accumulate via F=1 ones-column matmuls into a shared
PSUM tile; normalization is a per-partition reciprocal + tensor_scalar
multiply on DVE.  The normalized A [t, d] tiles are transposed back to
A^T [d, t] on the PE for the AllGather + output projection.

Scheduling: the kernel is paced by ACT (exp) once PE work is cut by fp8, so
block 0 runs "scores-first": both head pairs' scores/exp for all four
s-tiles are emitted before any V-projection or AV work, keeping ACT fed
from ~3us while the x/W bf16 DMAs land.  Later blocks use the woven s-loop
schedule (scores si+1 emitted before AV si; per-(head,t-tile) AV bursts at
the diagonal step; projections/output-projections as pipeline fillers, one
block earlier than their consumers).
"""

import numpy as np
from contextlib import ExitStack

import concourse.bass as bass
import concourse.mybir as mybir
import concourse.tile as tile
from concourse import bacc
from concourse.bass_utils import run_bass_kernel_spmd

F32 = mybir.dt.float32
BF16 = mybir.dt.bfloat16
FP8 = mybir.dt.float8e4
EXP = mybir.ActivationFunctionType.Exp
DR = mybir.MatmulPerfMode.DoubleRow

N_CORES = 8
B = 2
T = 2048
C = 1024
NH = 16
HS = 64
E = 1024
GROUPS = 4          # head groups (tensor-parallel ranks per batch)
HPG = NH // GROUPS  # 4 heads per core
ES = E // GROUPS    # 256 output columns per core
HD = HPG * HS       # 256 local attention-output rows

P = 128             # partition tile
TBLK = 512          # t-block (matmul moving dim)
NTB = T // TBLK     # 4
NCT = C // P        # 8 contraction tiles for projections
NST = T // P        # 16 key tiles

WSC = 64.0          # host-side fp8 scale on Wq/Wk
XSC = 4.0           # host-side fp8 scale on x
EXPSC = 0.125 / (WSC * XSC) ** 2   # exp(EXPSC * S_scaled) = exp(S / sqrt(hs))

REPLICA_GROUPS = [[0, 1, 2, 3], [4, 5, 6, 7]]


def build_nc(with_collective=True):
    """Build + compile the per-core SPMD program. Same program on all cores."""
    nc = bacc.Bacc(
        "TRN2", target_bir_lowering=False, debug=False, num_devices=N_CORES
    )

    xT = nc.dram_tensor("xT", [C, T], BF16, kind="ExternalInput").ap()
    x8T = nc.dram_tensor("x8", [C, T], FP8, kind="ExternalInput").ap()
    # fp8 weights arrive pre-shuffled to the SBUF image [p, (c d)] so the DMA
    # moves one contiguous 2KB run per partition
    wq8 = nc.dram_tensor("wq", [P, NCT * ES], FP8, kind="ExternalInput").ap()
    wk8 = nc.dram_tensor("wk", [P, NCT * ES], FP8, kind="ExternalInput").ap()
    wv = nc.dram_tensor("wv", [C, HD], BF16, kind="ExternalInput").ap()
    wo = nc.dram_tensor("wo", [E, ES], BF16, kind="ExternalInput").ap()
    # tri (cols 0:P), identity (cols P:2P), broadcast bias (cols 2P:2P+ES)
    trbo = nc.dram_tensor("trbo", [P, 2 * P + ES], BF16,
                          kind="ExternalInput").ap()
    out = nc.dram_tensor("out", [T, ES], BF16, kind="ExternalOutput").ap()

    with tile.TileContext(nc) as tc, ExitStack() as ctx:
        wp = ctx.enter_context(tc.tile_pool(name="wp", bufs=1))
        xp = ctx.enter_context(tc.tile_pool(name="xp", bufs=2))
        x8p = ctx.enter_context(tc.tile_pool(name="x8p", bufs=2))
        qkp = ctx.enter_context(tc.tile_pool(name="qkp", bufs=1))
        vp = ctx.enter_context(tc.tile_pool(name="vp", bufs=1))
        ptp = ctx.enter_context(tc.tile_pool(name="ptp", bufs=20))
        anp = ctx.enter_context(tc.tile_pool(name="anp", bufs=3))
        atp = ctx.enter_context(tc.tile_pool(name="atp", bufs=2))
        smp = ctx.enter_context(tc.tile_pool(name="smp", bufs=4))
        outp = ctx.enter_context(tc.tile_pool(name="outp", bufs=3))
        lhp = ctx.enter_context(tc.tile_pool(name="lhp", bufs=9))
        # PSUM: 8 banks.  st [128,1024] x2 bufs = 4 banks, attps 1, denps 1,
        # small (qkv-proj / oproj / transpose) 2.
        ps2 = ctx.enter_context(tc.tile_pool(name="ps2", bufs=2, space="PSUM"))
        psA = ctx.enter_context(tc.tile_pool(name="psA", bufs=1, space="PSUM"))
        psD = ctx.enter_context(tc.tile_pool(name="psD", bufs=1, space="PSUM"))
        psC = ctx.enter_context(tc.tile_pool(name="psC", bufs=2, space="PSUM"))
        dramp = ctx.enter_context(tc.tile_pool(name="dramp", bufs=1, space="DRAM"))

        # ---- small constants ----
        trbo_sb = wp.tile([P, 2 * P + ES], BF16, tag="trbo")
        tri_sb = trbo_sb[:, 0:P]
        id_sb = trbo_sb[:, P:2 * P]
        bias_sb = trbo_sb[:, 2 * P:2 * P + ES]
        onescol = wp.tile([P, 1], BF16, tag="onescol")
        nc.vector.memset(onescol[:], 1.0)

        # weights: one wide tile per tensor, chunk ci at cols [ci*ES, ...)
        w_sb = {
            "wq": wp.tile([P, NCT * ES], FP8, tag="w_wq", name="w_wq"),
            "wk": wp.tile([P, NCT * ES], FP8, tag="w_wk", name="w_wk"),
            "wv": wp.tile([P, NCT * ES], BF16, tag="w_wv", name="w_wv"),
            "wo": wp.tile([P, NCT * ES], BF16, tag="w_wo", name="w_wo"),
        }

        def wsl(name, ci):
            return w_sb[name][:, ci * ES:(ci + 1) * ES]

        def w8sl(name, i, pr):
            # chunk-pair i of the fp8 weights as a DoubleRow lhsT
            # [p, 2 k-tiles, 128 out-rows] for head pair pr
            return w_sb[name][:, 2 * i * ES:(2 * i + 2) * ES].rearrange(
                "p (c m) -> p c m", m=ES)[:, :, pr * P:(pr + 1) * P]

        def emit_w_dma(name, dram, ci0, nch, eng=None):
            (eng or nc.sync).dma_start(
                w_sb[name][:, ci0 * ES:(ci0 + nch) * ES].rearrange(
                    "p (c d) -> p c d", d=ES),
                dram.rearrange("(c p) d -> p c d", p=P)[:, ci0:ci0 + nch, :],
            )

        def emit_w8_dma(name, dram, c0, c1, eng=None):
            (eng or nc.sync).dma_start(w_sb[name][:, c0:c1], dram[:, c0:c1])

        # denominators: slice (pr, head-in-pair, tt) -> one fp32 column
        denps = psD.tile([P, 16], F32, tag="denps")

        # x^T per t-block (bf16 for V): chunk ci at cols [ci*TBLK, ...)
        xw_of = {}

        def xsl(tb, ci):
            return xw_of[tb][:, ci * TBLK:(ci + 1) * TBLK]

        # x fp8 per t-block (for Q/K): chunk-pair i as DoubleRow rhs
        x8_of = {}

        def x8sl(tb, i):
            return x8_of[tb][:, 2 * i * TBLK:(2 * i + 2) * TBLK].rearrange(
                "p (c t) -> p c t", t=TBLK)

        # merged Q^T/K^T per head pair: col = tb*1024 + qk*512 + t_local
        # (pair p holds heads 2p (rows 0-63) and 2p+1 (rows 64-127))
        qkt = [qkp.tile([P, 2 * T], BF16, tag=f"qk{p_}", name=f"qk{p_}")
               for p_ in range(2)]

        def qt_slice(pr, r0, rn, t0, tn):
            tb, tl = t0 // TBLK, t0 % TBLK
            base = tb * 1024 + tl
            return qkt[pr][r0:r0 + rn, base:base + tn]

        def kt_slice(pr, r0, rn, s0, sn):
            tb, sl = s0 // TBLK, s0 % TBLK
            base = tb * 1024 + TBLK + sl
            return qkt[pr][r0:r0 + rn, base:base + sn]

        v_sb = [vp.tile([P, HPG * HS], BF16, tag=f"v{st}", name=f"v{st}")
                for st in range(NST)]

        # ---------------- stage-1 pieces ----------------
        def emit_x_dma(tb, eng=None):
            ts_ = tb * TBLK
            xw_of[tb] = xp.tile([P, NCT * TBLK], BF16, tag="xw",
                                name=f"xw{tb}")
            (eng or nc.sync).dma_start(
                xw_of[tb][:].rearrange("p (c t) -> p c t", t=TBLK),
                xT.rearrange("(c p) t -> p c t", p=P)[:, :, ts_:ts_ + TBLK],
            )

        def emit_x8_dma(tb, ci0=0, nch=NCT, eng=None):
            ts_ = tb * TBLK
            if tb not in x8_of:
                x8_of[tb] = x8p.tile([P, NCT * TBLK], FP8, tag="x8",
                                     name=f"x8_{tb}")
            (eng or nc.sync).dma_start(
                x8_of[tb][:, ci0 * TBLK:(ci0 + nch) * TBLK].rearrange(
                    "p (c t) -> p c t", t=TBLK),
                x8T.rearrange("(c p) t -> p c t", p=P)[
                    :, ci0:ci0 + nch, ts_:ts_ + TBLK],
            )

        def emit_qk_proj(tb, pr, which):
            # fp8 DoubleRow: 4 chunk-pair matmuls cover all 8 c-chunks at
            # 0.5 cycles/row -> ~0.43us of PE per (tb, pr, which)
            wn = "wq" if which == 0 else "wk"
            ps = psC.tile([P, TBLK], F32, tag="small",
                          name=f"qkps{tb}_{pr}_{which}")
            for i in range(4):
                nc.tensor.matmul(
                    ps[:], lhsT=w8sl(wn, i, pr), rhs=x8sl(tb, i),
                    start=(i == 0), stop=(i == 3), perf_mode=DR,
                )
            base = tb * 1024 + which * TBLK
            nc.vector.tensor_copy(qkt[pr][:, base:base + TBLK], ps[:])

        def qk_units(tb):
            # 4 units per t-block: (pr0,Q), (pr0,K), (pr1,Q), (pr1,K)
            return [lambda pr=pr, w=w: emit_qk_proj(tb, pr, w)
                    for pr in range(2) for w in range(2)]

        def emit_v_proj(st):
            tb, sl = st // 4, (st % 4) * P
            vps = psC.tile([P, TBLK], F32, tag="small", name=f"vps{st}")
            for ci in range(NCT):
                nc.tensor.matmul(
                    vps[:, 0:HD],
                    lhsT=xsl(tb, ci)[:, sl:sl + P],
                    rhs=wsl("wv", ci),
                    start=(ci == 0), stop=(ci == NCT - 1),
                )
            nc.vector.tensor_copy(v_sb[st][:], vps[:, 0:HD])

        def v_chunks(tb):
            return [lambda st=st: emit_v_proj(st)
                    for st in range(4 * tb, 4 * tb + 4)]

        # ------- stage-2: one head PAIR of one t-block ----------------------
        def emit_headpair(qb, pr, at, hook_tt=None, defer_tp=True, ahead=1):
            """s-loop over key tiles; both heads of the pair per step.  AV is
            non-transposed: attps[t_p, (par,tt) 64-col slice] with per-slice
            fp32 denominator columns in denps.  Each (par,tt) slice finishes
            at its diagonal s-step -> finalize (normalize + transpose into
            `at`) is woven in right there.

            `ahead` scores/exp steps are emitted before the first AV burst
            (ahead=1 is the classic software pipeline: scores(si+1) before
            AV(si); block 0 uses ahead=4 to front-run every exp past the
            V-projection DMA wall).  One yield per scores step, then one
            yield per s-tile of the burst phase.  hook_tt, if given, is
            called after finalize(tt)."""
            t0 = qb * TBLK
            ns = 4 * (qb + 1)
            attps = psA.tile([P, 4 * P], F32, tag="attps",
                             name=f"attps{qb}_{pr}")

            def scores_exp(si):
                diag = si * P >= t0
                ka = si * P - t0 if diag else 0
                stp = ps2.tile([P, 2 * TBLK], F32, tag="st",
                               name=f"st{qb}_{pr}_{si}")
                for par in range(2):
                    r0 = par * HS
                    nc.tensor.matmul(
                        stp[:, par * TBLK + ka:(par + 1) * TBLK],
                        lhsT=kt_slice(pr, r0, HS, si * P, P),
                        rhs=qt_slice(pr, r0, HS, t0 + ka, TBLK - ka),
                        start=True, stop=True,
                    )
                pt = ptp.tile([P, 2 * TBLK], BF16, tag="pt",
                              name=f"pt{qb}_{pr}_{si}")
                if ka > 0:
                    s3 = stp[:].rearrange("p (r c) -> p r c", r=2)[:, :, ka:TBLK]
                    p3 = pt[:].rearrange("p (r c) -> p r c", r=2)[:, :, ka:TBLK]
                    nc.scalar.activation(p3, s3, EXP, scale=EXPSC)
                else:
                    nc.scalar.activation(pt[:], stp[:], EXP, scale=EXPSC)
                return pt

            pending = []  # deferred transpose closures (one s-step later)

            def emit_tp(tt, an):
                tp = psC.tile([P, TBLK], BF16, tag="small",
                              name=f"tp{qb}_{pr}_{tt}",
                              padded_shape=[P, 2 * TBLK])
                nc.tensor.transpose(tp[:, 0:P], an[:], id_sb[:])
                nc.vector.tensor_copy(at[:, tt * P:(tt + 1) * P],
                                      tp[:, 0:P])
                if hook_tt is not None:
                    hook_tt(tt)

            def burst(si, tt, par, lo=0, hi=None, close=True):
                # den group first: its F=1 matmuls finish almost instantly,
                # so the DVE reciprocal overlaps the AV burst
                h = 2 * pr + par
                sl = (par * 4 + tt) * HS
                dc = pr * 8 + par * 4 + tt
                hi_ = si if hi is None else hi
                for sj in range(lo, hi_ + 1):
                    lhs = pts[sj][
                        :, par * TBLK + tt * P:par * TBLK + (tt + 1) * P]
                    nc.tensor.matmul(
                        denps[:, dc:dc + 1],
                        lhsT=lhs, rhs=onescol[:],
                        start=(sj == 0), stop=(close and sj == hi_),
                    )
                for sj in range(lo, hi_ + 1):
                    lhs = pts[sj][
                        :, par * TBLK + tt * P:par * TBLK + (tt + 1) * P]
                    nc.tensor.matmul(
                        attps[:, sl:sl + HS],
                        lhsT=lhs,
                        rhs=v_sb[sj][:, h * HS:(h + 1) * HS],
                        start=(sj == 0), stop=(close and sj == hi_),
                    )

            def norm(tt, par, an):
                # normalize on DVE; par0's chain overlaps par1's PE burst
                sl = (par * 4 + tt) * HS
                dc = pr * 8 + par * 4 + tt
                rc = smp.tile([P, 1], F32, tag="rc")
                nc.vector.reciprocal(rc[:], denps[:, dc:dc + 1])
                nc.vector.tensor_scalar_mul(
                    an[:, par * HS:(par + 1) * HS],
                    attps[:, sl:sl + HS], rc[:])

            # PSUM accumulation groups must not interleave within a bank on
            # real hardware, so AV for each (head, t-tile, par) is ONE
            # contiguous matmul burst over all its s-tiles, emitted at that
            # t-tile's diagonal s-step; every pt tile of the pair stays
            # resident until its last burst.  attps and denps are separate
            # banks, so their per-(head,t-tile) groups may alternate.  The
            # two par bursts straddle a yield so a filler or the next scores
            # emission lands between them (keeps the exp cadence smooth).
            pts = []
            for i in range(ahead):
                pts.append(scores_exp(i))
                yield
            for si in range(ns):
                if si + ahead < ns:
                    pts.append(scores_exp(si + ahead))
                diag = si * P >= t0
                if diag:
                    tt = (si * P - t0) // P
                    for par in range(2):
                        c0 = par * TBLK + tt * P
                        nc.vector.tensor_mul(
                            pts[si][:, c0:c0 + P], pts[si][:, c0:c0 + P],
                            tri_sb[:])
                    burst(si, tt, 0)
                yield
                while pending:
                    pending.pop(0)()
                if not diag:
                    continue
                burst(si, tt, 1)
                an = anp.tile([P, P], BF16, tag="an",
                              name=f"an{qb}_{pr}_{tt}")
                for par in range(2):
                    norm(tt, par, an)
                if defer_tp:
                    pending.append(lambda tt=tt, an=an: emit_tp(tt, an))
                else:
                    emit_tp(tt, an)
                yield
            while pending:
                pending.pop(0)()

        # ---------------- stage-3: one t-tile of one t-block ---------------
        # lh_of[qb] = [lhw_pr0, lhw_pr1]: wide AllGathered A^T tiles, rank g
        # at cols [g*TBLK, (g+1)*TBLK).  hdt (wo row-chunk) = 2g + pr.
        osb_of = {}

        def lh_sl(qb, hdt, tt):
            return lh_of[qb][hdt % 2][:, (hdt // 2) * TBLK + tt * P:
                                      (hdt // 2) * TBLK + (tt + 1) * P]

        def emit_oproj_tt(qb, tt, half=None, holder=None):
            t0 = qb * TBLK
            if half == 1:
                op = holder["ps"]
            else:
                op = psC.tile([P, TBLK], F32, tag="small", name=f"op{qb}_{tt}")
                if holder is not None:
                    holder["ps"] = op
            # pr0 tiles (even hdt) first: they arrive one AllGather earlier
            order = [0, 2, 4, 6, 1, 3, 5, 7]
            sel = order if half is None else order[4 * half:4 * half + 4]
            for i, hdt in enumerate(sel):
                nc.tensor.matmul(
                    op[:, 0:ES],
                    lhsT=lh_sl(qb, hdt, tt),
                    rhs=wsl("wo", hdt),
                    start=(half != 1 and i == 0),
                    stop=((half is None or half == 1) and i == len(sel) - 1),
                )
            if half == 0:
                return
            if qb not in osb_of:
                osb_of[qb] = (outp.tile([P, 4 * ES], BF16, tag="osb",
                                        name=f"osb{qb}"), [])
            osb, done = osb_of[qb]
            nc.vector.tensor_add(
                osb[:, tt * ES:(tt + 1) * ES], op[:, 0:ES], bias_sb[:])
            done.append(tt)
            if len(done) == 4:
                nc.sync.dma_start(
                    out[t0:t0 + TBLK, :].rearrange("(s p) e -> p s e", p=P),
                    osb[:].rearrange("p (s e) -> p s e", e=ES),
                )

        # --------- per-pair AllGather (pr = head pair 0/1 of this core) -----
        def emit_ag_cols(key, at, c0, cn):
            """Gather at[:, c0:c0+cn] across the 4 ranks of this batch;
            returns the wide SBUF tile with rank g at cols [g*cn, (g+1)*cn)."""
            ag_out = dramp.tile([GROUPS * P, cn], BF16, tag=f"agout{key}",
                                name=f"agout{key}")
            if with_collective:
                ag_in = dramp.tile([P, cn], BF16, tag=f"agin{key}",
                                   name=f"agin{key}")
                nc.sync.dma_start(ag_in[:], at[:, c0:c0 + cn])
                nc.gpsimd.collective_compute(
                    "AllGather",
                    mybir.AluOpType.bypass,
                    replica_groups=REPLICA_GROUPS,
                    ins=[ag_in[:].opt()],
                    outs=[ag_out[:].opt()],
                )
            else:
                # timing/sim variant: model the collective's local footprint
                # (own-contribution write; peer slots arrive over D2D, which
                # doesn't occupy the local DMA engines)
                nc.sync.dma_start(ag_out[0:P, :], at[:, c0:c0 + cn])
            lhw = lhp.tile([P, GROUPS * cn], BF16, tag="lh",
                           name=f"lh{key}")
            nc.sync.dma_start(
                lhw[:].rearrange("p (g t) -> p g t", g=GROUPS),
                ag_out[:].rearrange("(g p) t -> p g t", g=GROUPS),
            )
            return lhw

        def emit_ag(qb, pr, at):
            lh_of[qb][pr] = emit_ag_cols(f"{qb}_{pr}", at, 0, TBLK)

        # ---------------- emission schedule ----------------
        # startup DMAs in bus-priority order across two HWDGE queues; the
        # first Q/K DoubleRow matmul needs only wq8/x8(0) chunk-pair 0
        emit_w8_dma("wq", wq8, 0, NCT * ES)         # sync
        emit_x8_dma(0, 0, 2, eng=nc.scalar)         # scalar
        emit_w8_dma("wk", wk8, 0, NCT * ES)         # sync
        emit_x8_dma(0, 2, 6, eng=nc.scalar)         # scalar
        emit_x8_dma(1, eng=nc.sync)
        emit_x_dma(0, eng=nc.scalar)                # bf16, for V
        nc.sync.dma_start(trbo_sb[:], trbo[:])
        emit_w_dma("wv", wv, 0, NCT, eng=nc.scalar)
        emit_x_dma(1, eng=nc.sync)
        emit_w_dma("wo", wo, 0, NCT, eng=nc.scalar)

        def drive(gen, vfill, fillers, ns, vstart=None, pre_gen=None,
                  pre_at=(), at=None):
            """Drive a head pair's s-loop.  V fillers pop at ctrs
            vstart+1.. (early-mid loop, after their xT DMA but before their
            diagonal-step deadlines); generic fillers are spread evenly
            (Bresenham) over the remaining slots; pre_gen (the next pair) is
            advanced one step at each ctr in pre_at so its scores/exp keep
            ACT fed across the pair transition; `at` maps specific ctrs to
            extra closures (used to pin late work after a hook has fired)."""
            ctr = 0
            nf = len(fillers)
            done = 0
            vpops = 0
            if vstart is None:
                vstart = ns - len(vfill)
            den = max(1, ns - 2 - len(vfill))
            for _ in gen:
                ctr += 1
                if pre_gen is not None and ctr in pre_at:
                    next(pre_gen)
                if at and ctr in at:
                    for fn in at[ctr]:
                        fn()
                if vfill and ctr > vstart:
                    vfill.pop(0)()
                    vpops += 1
                else:
                    target = -(-(ctr - vpops) * nf // den)  # ceil
                    while done < target and fillers:
                        fillers.pop(0)()
                        done += 1

        lh_of = {qb: [None, None] for qb in range(NTB)}

        def new_at(qb, pr):
            return atp.tile([P, TBLK], BF16, tag="at", name=f"at{qb}_{pr}")

        def oproj_fillers(qb):
            # half-units (~0.45us) pack into sub-us pipeline holes
            units = []
            for tt in range(4):
                holder = {}
                units.append(lambda tt=tt, q=qb, h=holder:
                             emit_oproj_tt(q, tt, 0, h))
                units.append(lambda tt=tt, q=qb, h=holder:
                             emit_oproj_tt(q, tt, 1, h))
            return units

        def drain(lst):
            while lst:
                lst.pop(0)()

        def drain_gen(g):
            for _ in g:
                pass

        # ---- t-block 0: scores-first ----
        # PE order: p0 Q/K proj, all 8 scores/exp of both pairs (p1 proj and
        # block-1 p0 proj woven), then V + AV bursts with block-1 scores
        # trickling in via pre_at.  ACT runs exp back to back from ~5us
        # while xT/wv stream in for the V projections.
        qk0 = qk_units(0)
        qk1 = qk_units(1)
        qk0[0]()            # p0 Q
        qk0[1]()            # p0 K
        at00 = new_at(0, 0)
        at01 = new_at(0, 1)
        at10 = new_at(1, 0)
        g00 = emit_headpair(0, 0, at00, ahead=4)
        g01 = emit_headpair(0, 1, at01, ahead=4)
        g10 = emit_headpair(1, 0, at10, ahead=4)
        next(g00)           # s00_0
        next(g00)           # s00_1
        qk0[2]()            # p1 Q
        next(g00)           # s00_2
        qk0[3]()            # p1 K
        next(g00)           # s00_3
        for _ in range(4):
            next(g01)       # s01_0..3
        qk1[0]()            # block-1 p0 Q
        qk1[1]()            # block-1 p0 K
        v1 = v_chunks(1)
        next(g10)           # s10_0
        next(g10)           # s10_1  (ahead of the xT/wv DMA wall)
        v0 = v_chunks(0)
        v0.pop(0)()         # V(0) ahead of the first diagonal burst
        drive(g00, v0, [], 8, vstart=1, pre_gen=g10, pre_at=(3, 5))
        emit_ag(0, 0, at00)
        emit_x8_dma(2)
        emit_x_dma(2)
        drive(g01, v1[1:4], qk1[2:], 8, vstart=5, pre_gen=g10,
              pre_at=(1, 3, 5, 7))
        emit_ag(0, 1, at01)
        del v1[1:4]

        # ---- t-block 1 ----
        qk2 = qk_units(2)
        at11 = new_at(1, 1)
        g11 = emit_headpair(1, 1, at11, ahead=2)
        v1.pop(0)()         # V(4) ahead of g10's first diagonal burst
        drive(g10, v1, qk2[:2], 8, vstart=1, pre_gen=g11,
              pre_at=(1, 3, 5, 7))
        emit_ag(1, 0, at10)
        emit_x8_dma(3)
        emit_x_dma(3)
        opr0 = oproj_fillers(0)
        at20 = new_at(2, 0)
        g20 = emit_headpair(2, 0, at20, ahead=2)
        drive(g11, [], qk2[2:] + opr0[:4], 10, pre_gen=g20,
              pre_at=(3, 5, 7, 9))
        emit_ag(1, 1, at11)

        # ---- t-block 2 ----
        v2 = v_chunks(2)
        opr1 = oproj_fillers(1)
        at21 = new_at(2, 1)
        g21 = emit_headpair(2, 1, at21, ahead=2)
        drive(g20, v2, opr0[4:6], 14, vstart=2,
              pre_gen=g21, pre_at=(7, 9, 11, 13))
        emit_ag(2, 0, at20)
        qk3 = qk_units(3)
        at30 = new_at(3, 0)
        g30 = emit_headpair(3, 0, at30, ahead=2)
        drive(g21, [], qk3 + opr0[6:] + opr1[:2], 14, pre_gen=g30,
              pre_at=(7, 9, 11, 13))
        emit_ag(2, 1, at21)

        # ---- t-block 3 ----
        v3 = v_chunks(3)
        opr2 = oproj_fillers(2)
        at31 = new_at(3, 1)
        lh31 = {}
        # tt0+tt1 gather as one half; tt2 and tt3 each gather alone so the
        # final output projection only ever waits on the 128-col tile that
        # actually finished last
        agw = [2 * P, None, P, P]
        ag31out = {tt: dramp.tile([GROUPS * P, agw[tt]], BF16,
                                  tag=f"agout31{tt}", name=f"agout31{tt}")
                   for tt in (0, 2, 3)}
        ag31in = {tt: dramp.tile([P, agw[tt]], BF16, tag=f"agin31{tt}",
                                 name=f"agin31{tt}")
                  for tt in (0, 2, 3)}

        def hook31(tt):
            # stage each finished t-tile into its collective input as soon
            # as it exists; gathers ride the idle scalar queue
            key = 0 if tt < 2 else tt
            col = (tt % 2) * P if tt < 2 else 0
            src = at31[:, tt * P:(tt + 1) * P]
            stage = ag31in[key] if with_collective else ag31out[key]
            nc.scalar.dma_start(stage[0:P, col:col + P], src)
            if tt == 0:
                return
            if with_collective:
                nc.gpsimd.collective_compute(
                    "AllGather",
                    mybir.AluOpType.bypass,
                    replica_groups=REPLICA_GROUPS,
                    ins=[ag31in[key][:].opt()],
                    outs=[ag31out[key][:].opt()],
                )
            lhw = lhp.tile([P, GROUPS * agw[key]], BF16, tag="lh",
                           name=f"lh31_{key}")
            nc.scalar.dma_start(
                lhw[:].rearrange("p (g t) -> p g t", g=GROUPS),
                ag31out[key][:].rearrange("(g p) t -> p g t", g=GROUPS),
            )
            lh31[key] = lhw

        # final-block out-projection: one complete 8-matmul chain per t-tile
        # (pr-0 hd-tiles from AG(3,0), pr-1 from the half-AGs; in lh31
        # halves rank g sits at cols [g*256,+256), tt%2 picks the 128-col
        # t-tile).  The op accumulators live in the ps2 (scores) pool,
        # which is free once the last scores are emitted -- this keeps the
        # tail off the small-psum pool and its transpose-DMA WAR chains.
        # tt0/tt1 are injected into g31's loop right after the first
        # half-AllGather fires (via drive's `at`); tt2/tt3 follow the loop.
        tz = (NTB - 1) * TBLK
        osbz = outp.tile([P, 4 * ES], BF16, tag="osb", name="osbz")
        opz = [None]

        def emit_tz_tt(tt):
            if opz[0] is None:
                opz[0] = ps2.tile([P, 2 * TBLK], F32, tag="st", name="opz")
            op = opz[0][:, tt * ES:(tt + 1) * ES]
            key = 0 if tt < 2 else tt
            half = lh31[key]
            w = 2 * P if tt < 2 else P
            col = (tt % 2) * P if tt < 2 else 0
            for i, hdt in enumerate((0, 2, 4, 6)):
                nc.tensor.matmul(
                    op, lhsT=lh_sl(3, hdt, tt), rhs=wsl("wo", hdt),
                    start=(i == 0), stop=(i == 3),
                )
            for j, hdt in enumerate((1, 3, 5, 7)):
                g_ = (hdt - 1) // 2
                nc.tensor.matmul(
                    op,
                    lhsT=half[:, g_ * w + col:g_ * w + col + P],
                    rhs=wsl("wo", hdt),
                    start=False, stop=(j == 3),
                )
            nc.vector.tensor_add(
                osbz[:, tt * ES:(tt + 1) * ES], op, bias_sb[:])
            nc.sync.dma_start(
                out[tz + tt * P:tz + (tt + 1) * P, :],
                osbz[:, tt * ES:(tt + 1) * ES],
            )

        # last pair: transposes NOT deferred, so the per-t-tile staging DMAs
        # fire as early as possible
        g31 = emit_headpair(3, 1, at31, hook_tt=hook31, defer_tp=False,
                            ahead=2)
        drive(g30, v3, opr1[2:], 18, vstart=6, pre_gen=g31,
              pre_at=(11, 13, 15, 17))
        emit_ag(3, 0, at30)
        drive(g31, [], opr2, 14,
              at={15: [lambda: emit_tz_tt(0)], 17: [lambda: emit_tz_tt(1)]})
        drain(opr2)
        emit_tz_tt(2)
        emit_tz_tt(3)

    nc.compile()
    return nc


_NC_CACHE = {}


def _get_nc(with_collective=True):
    key = with_collective
    if key not in _NC_CACHE:
        _NC_CACHE[key] = build_nc(with_collective)
    return _NC_CACHE[key]


def make_in_maps(x, Wq, Wk, Wv, Wo, bo):
    import ml_dtypes
    bf16 = ml_dtypes.bfloat16
    f8 = ml_dtypes.float8_e4m3
    trbo = np.concatenate(
        [np.triu(np.ones((P, P), dtype=np.float32)),
         np.eye(P, dtype=np.float32),
         np.zeros((P, ES), dtype=np.float32)], axis=1)
    in_maps = []
    for c in range(N_CORES):
        b, g = c // GROUPS, c % GROUPS
        hs_ = slice(g * HPG, (g + 1) * HPG)
        tb = trbo.copy()
        tb[:, 2 * P:] = bo[g * ES:(g + 1) * ES][None, :]

        def shuffle8(W):
            # [C, HD] -> SBUF image [P, NCT*HD] (chunk ci at cols ci*HD)
            w = (W[hs_].transpose(1, 0, 2).reshape(C, HD) * WSC)
            return np.ascontiguousarray(
                w.reshape(NCT, P, HD).transpose(1, 0, 2).reshape(
                    P, NCT * HD)).astype(f8)

        in_maps.append({
            "xT": np.ascontiguousarray(x[b].T).astype(bf16),
            "x8": np.ascontiguousarray(x[b].T * XSC).astype(f8),
            "wq": shuffle8(Wq),
            "wk": shuffle8(Wk),
            "wv": np.ascontiguousarray(
                Wv[hs_].transpose(1, 0, 2).reshape(C, HD)).astype(bf16),
            "wo": np.ascontiguousarray(
                Wo[:, g * ES:(g + 1) * ES]).astype(bf16),
            "trbo": np.ascontiguousarray(tb).astype(bf16),
        })
    return in_maps


def kernel(x, Wq, Wk, Wv, Wo, bo):
    x = np.asarray(x, dtype=np.float32)
    Wq = np.asarray(Wq, dtype=np.float32)
    Wk = np.asarray(Wk, dtype=np.float32)
    Wv = np.asarray(Wv, dtype=np.float32)
    Wo = np.asarray(Wo, dtype=np.float32)
    bo = np.asarray(bo, dtype=np.float32)

    nc = _get_nc(with_collective=True)
    in_maps = make_in_maps(x, Wq, Wk, Wv, Wo, bo)
    res = run_bass_kernel_spmd(nc, in_maps, core_ids=list(range(N_CORES)))

    out = np.empty((B, T, E), dtype=np.float32)
    for c in range(N_CORES):
        b, g = c // GROUPS, c % GROUPS
        out[b, :, g * ES:(g + 1) * ES] = np.asarray(
            res.results[c]["out"], dtype=np.float32)
    return out


# revision 25
# speedup vs baseline: 1.0008x; 1.0008x over previous
"""Multi-head causal self-attention on 8 TRN2 NeuronCores (bf16 + fp8 QK).

Problem (nn_MultiHeadAttention): B=2, T=2048, C=1024, H=16 heads, hs=64.
  q,k,v = per-head projections of x; causal softmax(q k^T / 8) v;
  concat heads; out = att @ Wo + bo.

Sharding: core c in 0..7 -> (batch b = c//4, head-group g = c%4, 4 heads each).
Each core computes Q/K/V + causal attention for its 4 heads on its batch,
normalized attention outputs are AllGathered across the 4 cores of the same
batch (replica groups [0-3], [4-7]), then each core computes a disjoint
256-column slice of the output projection (column-parallel Wo) + bias slice.
Host does a pure concat of the 8 disjoint output slices.

Q/K projections run in fp8e4 DoubleRow mode: W (x64) and x (x4) are quantized
host-side, the PE contracts TWO 128-row c-chunks per pass at 0.5 cycles/row
(4x the bf16 rate), and the combined 2^16 scale is folded into the softmax
exp scale (0.125 / 65536).  Quantization error enters only through the
softmax logits, where it attenuates to ~1% of the output.  V, the attention
matmuls, and the output projection stay bf16 (their error hits the output
directly); PSUM accumulation is fp32 throughout.

Attention layout: scores stay transposed (S^T [s, t], lhsT=K^T, rhs=Q^T),
exp runs on ACT into bf16 P^T tiles, and AV is computed non-transposed:
out[t_p, d_f] += P^T[s, t-tile].T @ V[s, d] with full 128 output partitions.
Softmax denominators accumulate via F=1 ones-column matmuls into a shared
PSUM tile; normalization is a per-partition reciprocal + tensor_scalar
multiply on DVE.  The normalized A [t, d] tiles are transposed back to
A^T [d, t] on the PE for the AllGather + output projection.

Scheduling: the kernel is paced by ACT (exp) once PE work is cut by fp8, so
block 0 runs "scores-first": both head pairs' scores/exp for all four
s-tiles are emitted before any V-projection or AV work, keeping ACT fed
from ~3us while the x/W bf16 DMAs land.  Later blocks use the woven s-loop
schedule (scores si+1 emitted before AV si; per-(head,t-tile) AV bursts at
the diagonal step; projections/output-projections as pipeline fillers, one
block earlier than their consumers).
"""

import numpy as np
from contextlib import ExitStack

import concourse.bass as bass
import concourse.mybir as mybir
import concourse.tile as tile
from concourse import bacc
from concourse.bass_utils import run_bass_kernel_spmd

F32 = mybir.dt.float32
BF16 = mybir.dt.bfloat16
FP8 = mybir.dt.float8e4
EXP = mybir.ActivationFunctionType.Exp
DR = mybir.MatmulPerfMode.DoubleRow

N_CORES = 8
B = 2
T = 2048
C = 1024
NH = 16
HS = 64
E = 1024
GROUPS = 4          # head groups (tensor-parallel ranks per batch)
HPG = NH // GROUPS  # 4 heads per core
ES = E // GROUPS    # 256 output columns per core
HD = HPG * HS       # 256 local attention-output rows

P = 128             # partition tile
TBLK = 512          # t-block (matmul moving dim)
NTB = T // TBLK     # 4
NCT = C // P        # 8 contraction tiles for projections
NST = T // P        # 16 key tiles

WSC = 64.0          # host-side fp8 scale on Wq/Wk
XSC = 4.0           # host-side fp8 scale on x
EXPSC = 0.125 / (WSC * XSC) ** 2   # exp(EXPSC * S_scaled) = exp(S / sqrt(hs))

REPLICA_GROUPS = [[0, 1, 2, 3], [4, 5, 6, 7]]


def build_nc(with_collective=True):
    """Build + compile the per-core SPMD program. Same program on all cores."""
    nc = bacc.Bacc(
        "TRN2", target_bir_lowering=False, debug=False, num_devices=N_CORES
    )

    xT = nc.dram_tensor("xT", [C, T], BF16, kind="ExternalInput").ap()
    x8T = nc.dram_tensor("x8", [C, T], FP8, kind="ExternalInput").ap()
    # fp8 weights arrive pre-shuffled to the SBUF image [p, (c d)] so the DMA
    # moves one contiguous 2KB run per partition
    wq8 = nc.dram_tensor("wq", [P, NCT * ES], FP8, kind="ExternalInput").ap()
    wk8 = nc.dram_tensor("wk", [P, NCT * ES], FP8, kind="ExternalInput").ap()
    wv = nc.dram_tensor("wv", [C, HD], BF16, kind="ExternalInput").ap()
    wo = nc.dram_tensor("wo", [E, ES], BF16, kind="ExternalInput").ap()
    # tri (cols 0:P), identity (cols P:2P), broadcast bias (cols 2P:2P+ES)
    trbo = nc.dram_tensor("trbo", [P, 2 * P + ES], BF16,
                          kind="ExternalInput").ap()
    out = nc.dram_tensor("out", [T, ES], BF16, kind="ExternalOutput").ap()

    with tile.TileContext(nc) as tc, ExitStack() as ctx:
        wp = ctx.enter_context(tc.tile_pool(name="wp", bufs=1))
        xp = ctx.enter_context(tc.tile_pool(name="xp", bufs=3))
        x8p = ctx.enter_context(tc.tile_pool(name="x8p", bufs=3))
        qkp = ctx.enter_context(tc.tile_pool(name="qkp", bufs=1))
        vp = ctx.enter_context(tc.tile_pool(name="vp", bufs=1))
        ptp = ctx.enter_context(tc.tile_pool(name="ptp", bufs=22))
        anp = ctx.enter_context(tc.tile_pool(name="anp", bufs=4))
        atp = ctx.enter_context(tc.tile_pool(name="atp", bufs=3))
        smp = ctx.enter_context(tc.tile_pool(name="smp", bufs=6))
        outp = ctx.enter_context(tc.tile_pool(name="outp", bufs=4))
        lhp = ctx.enter_context(tc.tile_pool(name="lhp", bufs=10))
        # PSUM: 8 banks.  st [128,1024] x2 bufs = 4 banks, attps 1, denps 1,
        # small (qkv-proj / oproj / transpose) 2.
        ps2 = ctx.enter_context(tc.tile_pool(name="ps2", bufs=2, space="PSUM"))
        psA = ctx.enter_context(tc.tile_pool(name="psA", bufs=1, space="PSUM"))
        psD = ctx.enter_context(tc.tile_pool(name="psD", bufs=1, space="PSUM"))
        psC = ctx.enter_context(tc.tile_pool(name="psC", bufs=2, space="PSUM"))
        dramp = ctx.enter_context(tc.tile_pool(name="dramp", bufs=1, space="DRAM"))

        # ---- small constants ----
        trbo_sb = wp.tile([P, 2 * P + ES], BF16, tag="trbo")
        tri_sb = trbo_sb[:, 0:P]
        id_sb = trbo_sb[:, P:2 * P]
        bias_sb = trbo_sb[:, 2 * P:2 * P + ES]
        onescol = wp.tile([P, 1], BF16, tag="onescol")
        nc.vector.memset(onescol[:], 1.0)

        # weights: one wide tile per tensor, chunk ci at cols [ci*ES, ...)
        w_sb = {
            "wq": wp.tile([P, NCT * ES], FP8, tag="w_wq", name="w_wq"),
            "wk": wp.tile([P, NCT * ES], FP8, tag="w_wk", name="w_wk"),
            "wv": wp.tile([P, NCT * ES], BF16, tag="w_wv", name="w_wv"),
            "wo": wp.tile([P, NCT * ES], BF16, tag="w_wo", name="w_wo"),
        }

        def wsl(name, ci):
            return w_sb[name][:, ci * ES:(ci + 1) * ES]

        def w8sl(name, i, pr):
            # chunk-pair i of the fp8 weights as a DoubleRow lhsT
            # [p, 2 k-tiles, 128 out-rows] for head pair pr
            return w_sb[name][:, 2 * i * ES:(2 * i + 2) * ES].rearrange(
                "p (c m) -> p c m", m=ES)[:, :, pr * P:(pr + 1) * P]

        def emit_w_dma(name, dram, ci0, nch, eng=None):
            (eng or nc.sync).dma_start(
                w_sb[name][:, ci0 * ES:(ci0 + nch) * ES].rearrange(
                    "p (c d) -> p c d", d=ES),
                dram.rearrange("(c p) d -> p c d", p=P)[:, ci0:ci0 + nch, :],
            )

        def emit_w8_dma(name, dram, c0, c1, eng=None):
            (eng or nc.sync).dma_start(w_sb[name][:, c0:c1], dram[:, c0:c1])

        # denominators: slice (pr, head-in-pair, tt) -> one fp32 column
        denps = psD.tile([P, 16], F32, tag="denps")

        # x^T per t-block (bf16 for V): chunk ci at cols [ci*TBLK, ...)
        xw_of = {}

        def xsl(tb, ci):
            return xw_of[tb][:, ci * TBLK:(ci + 1) * TBLK]

        # x fp8 per t-block (for Q/K): chunk-pair i as DoubleRow rhs
        x8_of = {}

        def x8sl(tb, i):
            return x8_of[tb][:, 2 * i * TBLK:(2 * i + 2) * TBLK].rearrange(
                "p (c t) -> p c t", t=TBLK)

        # merged Q^T/K^T per head pair: col = tb*1024 + qk*512 + t_local
        # (pair p holds heads 2p (rows 0-63) and 2p+1 (rows 64-127))
        qkt = [qkp.tile([P, 2 * T], BF16, tag=f"qk{p_}", name=f"qk{p_}")
               for p_ in range(2)]

        def qt_slice(pr, r0, rn, t0, tn):
            tb, tl = t0 // TBLK, t0 % TBLK
            base = tb * 1024 + tl
            return qkt[pr][r0:r0 + rn, base:base + tn]

        def kt_slice(pr, r0, rn, s0, sn):
            tb, sl = s0 // TBLK, s0 % TBLK
            base = tb * 1024 + TBLK + sl
            return qkt[pr][r0:r0 + rn, base:base + sn]

        v_sb = [vp.tile([P, HPG * HS], BF16, tag=f"v{st}", name=f"v{st}")
                for st in range(NST)]

        # ---------------- stage-1 pieces ----------------
        def emit_x_dma(tb, eng=None):
            ts_ = tb * TBLK
            xw_of[tb] = xp.tile([P, NCT * TBLK], BF16, tag="xw",
                                name=f"xw{tb}")
            (eng or nc.sync).dma_start(
                xw_of[tb][:].rearrange("p (c t) -> p c t", t=TBLK),
                xT.rearrange("(c p) t -> p c t", p=P)[:, :, ts_:ts_ + TBLK],
            )

        def emit_x8_dma(tb, ci0=0, nch=NCT, eng=None):
            ts_ = tb * TBLK
            if tb not in x8_of:
                x8_of[tb] = x8p.tile([P, NCT * TBLK], FP8, tag="x8",
                                     name=f"x8_{tb}")
            (eng or nc.sync).dma_start(
                x8_of[tb][:, ci0 * TBLK:(ci0 + nch) * TBLK].rearrange(
                    "p (c t) -> p c t", t=TBLK),
                x8T.rearrange("(c p) t -> p c t", p=P)[
                    :, ci0:ci0 + nch, ts_:ts_ + TBLK],
            )

        def emit_qk_proj(tb, pr, which):
            # fp8 DoubleRow: 4 chunk-pair matmuls cover all 8 c-chunks at
            # 0.5 cycles/row -> ~0.43us of PE per (tb, pr, which)
            wn = "wq" if which == 0 else "wk"
            ps = psC.tile([P, TBLK], F32, tag="small",
                          name=f"qkps{tb}_{pr}_{which}")
            for i in range(4):
                nc.tensor.matmul(
                    ps[:], lhsT=w8sl(wn, i, pr), rhs=x8sl(tb, i),
                    start=(i == 0), stop=(i == 3), perf_mode=DR,
                )
            base = tb * 1024 + which * TBLK
            nc.vector.tensor_copy(qkt[pr][:, base:base + TBLK], ps[:])

        def qk_units(tb):
            # 4 units per t-block: (pr0,Q), (pr0,K), (pr1,Q), (pr1,K)
            return [lambda pr=pr, w=w: emit_qk_proj(tb, pr, w)
                    for pr in range(2) for w in range(2)]

        def emit_v_proj(st):
            tb, sl = st // 4, (st % 4) * P
            vps = psC.tile([P, TBLK], F32, tag="small", name=f"vps{st}")
            for ci in range(NCT):
                nc.tensor.matmul(
                    vps[:, 0:HD],
                    lhsT=xsl(tb, ci)[:, sl:sl + P],
                    rhs=wsl("wv", ci),
                    start=(ci == 0), stop=(ci == NCT - 1),
                )
            nc.vector.tensor_copy(v_sb[st][:], vps[:, 0:HD])

        def v_chunks(tb):
            return [lambda st=st: emit_v_proj(st)
                    for st in range(4 * tb, 4 * tb + 4)]

        # ------- stage-2: one head PAIR of one t-block ----------------------
        def emit_headpair(qb, pr, at, hook_tt=None, defer_tp=True, ahead=1):
            """s-loop over key tiles; both heads of the pair per step.  AV is
            non-transposed: attps[t_p, (par,tt) 64-col slice] with per-slice
            fp32 denominator columns in denps.  Each (par,tt) slice finishes
            at its diagonal s-step -> finalize (normalize + transpose into
            `at`) is woven in right there.

            `ahead` scores/exp steps are emitted before the first AV burst
            (ahead=1 is the classic software pipeline: scores(si+1) before
            AV(si); block 0 uses ahead=4 to front-run every exp past the
            V-projection DMA wall).  One yield per scores step, then one
            yield per s-tile of the burst phase.  hook_tt, if given, is
            called after finalize(tt)."""
            t0 = qb * TBLK
            ns = 4 * (qb + 1)
            attps = psA.tile([P, 4 * P], F32, tag="attps",
                             name=f"attps{qb}_{pr}")

            def scores_exp(si):
                diag = si * P >= t0
                ka = si * P - t0 if diag else 0
                stp = ps2.tile([P, 2 * TBLK], F32, tag="st",
                               name=f"st{qb}_{pr}_{si}")
                for par in range(2):
                    r0 = par * HS
                    nc.tensor.matmul(
                        stp[:, par * TBLK + ka:(par + 1) * TBLK],
                        lhsT=kt_slice(pr, r0, HS, si * P, P),
                        rhs=qt_slice(pr, r0, HS, t0 + ka, TBLK - ka),
                        start=True, stop=True,
                    )
                pt = ptp.tile([P, 2 * TBLK], BF16, tag="pt",
                              name=f"pt{qb}_{pr}_{si}")
                if ka > 0:
                    s3 = stp[:].rearrange("p (r c) -> p r c", r=2)[:, :, ka:TBLK]
                    p3 = pt[:].rearrange("p (r c) -> p r c", r=2)[:, :, ka:TBLK]
                    nc.scalar.activation(p3, s3, EXP, scale=EXPSC)
                else:
                    nc.scalar.activation(pt[:], stp[:], EXP, scale=EXPSC)
                return pt

            pending = []  # deferred transpose closures (one s-step later)

            def emit_tp(tt, an):
                tp = psC.tile([P, TBLK], BF16, tag="small",
                              name=f"tp{qb}_{pr}_{tt}",
                              padded_shape=[P, 2 * TBLK])
                nc.tensor.transpose(tp[:, 0:P], an[:], id_sb[:])
                nc.vector.tensor_copy(at[:, tt * P:(tt + 1) * P],
                                      tp[:, 0:P])
                if hook_tt is not None:
                    hook_tt(tt)

            def burst(si, tt, par, lo=0, hi=None, close=True):
                # den group first: its F=1 matmuls finish almost instantly,
                # so the DVE reciprocal overlaps the AV burst
                h = 2 * pr + par
                sl = (par * 4 + tt) * HS
                dc = pr * 8 + par * 4 + tt
                hi_ = si if hi is None else hi
                for sj in range(lo, hi_ + 1):
                    lhs = pts[sj][
                        :, par * TBLK + tt * P:par * TBLK + (tt + 1) * P]
                    nc.tensor.matmul(
                        denps[:, dc:dc + 1],
                        lhsT=lhs, rhs=onescol[:],
                        start=(sj == 0), stop=(close and sj == hi_),
                    )
                for sj in range(lo, hi_ + 1):
                    lhs = pts[sj][
                        :, par * TBLK + tt * P:par * TBLK + (tt + 1) * P]
                    nc.tensor.matmul(
                        attps[:, sl:sl + HS],
                        lhsT=lhs,
                        rhs=v_sb[sj][:, h * HS:(h + 1) * HS],
                        start=(sj == 0), stop=(close and sj == hi_),
                    )

            def norm(tt, par, an):
                # normalize on DVE; par0's chain overlaps par1's PE burst
                sl = (par * 4 + tt) * HS
                dc = pr * 8 + par * 4 + tt
                rc = smp.tile([P, 1], F32, tag="rc")
                nc.vector.reciprocal(rc[:], denps[:, dc:dc + 1])
                nc.vector.tensor_scalar_mul(
                    an[:, par * HS:(par + 1) * HS],
                    attps[:, sl:sl + HS], rc[:])

            # PSUM accumulation groups must not interleave within a bank on
            # real hardware, so AV for each (head, t-tile, par) is ONE
            # contiguous matmul burst over all its s-tiles, emitted at that
            # t-tile's diagonal s-step; every pt tile of the pair stays
            # resident until its last burst.  attps and denps are separate
            # banks, so their per-(head,t-tile) groups may alternate.  The
            # two par bursts straddle a yield so a filler or the next scores
            # emission lands between them (keeps the exp cadence smooth).
            pts = []
            for i in range(ahead):
                pts.append(scores_exp(i))
                yield
            for si in range(ns):
                if si + ahead < ns:
                    pts.append(scores_exp(si + ahead))
                diag = si * P >= t0
                if diag:
                    tt = (si * P - t0) // P
                    for par in range(2):
                        c0 = par * TBLK + tt * P
                        nc.vector.tensor_mul(
                            pts[si][:, c0:c0 + P], pts[si][:, c0:c0 + P],
                            tri_sb[:])
                    burst(si, tt, 0)
                yield
                while pending:
                    pending.pop(0)()
                if not diag:
                    continue
                burst(si, tt, 1)
                an = anp.tile([P, P], BF16, tag="an",
                              name=f"an{qb}_{pr}_{tt}")
                for par in range(2):
                    norm(tt, par, an)
                if defer_tp:
                    pending.append(lambda tt=tt, an=an: emit_tp(tt, an))
                else:
                    emit_tp(tt, an)
                yield
            while pending:
                pending.pop(0)()

        # ---------------- stage-3: one t-tile of one t-block ---------------
        # lh_of[qb] = [lhw_pr0, lhw_pr1]: wide AllGathered A^T tiles, rank g
        # at cols [g*TBLK, (g+1)*TBLK).  hdt (wo row-chunk) = 2g + pr.
        osb_of = {}

        def lh_sl(qb, hdt, tt):
            return lh_of[qb][hdt % 2][:, (hdt // 2) * TBLK + tt * P:
                                      (hdt // 2) * TBLK + (tt + 1) * P]

        def emit_oproj_tt(qb, tt, half=None, holder=None):
            t0 = qb * TBLK
            if half == 1:
                op = holder["ps"]
            else:
                op = psC.tile([P, TBLK], F32, tag="small", name=f"op{qb}_{tt}")
                if holder is not None:
                    holder["ps"] = op
            # pr0 tiles (even hdt) first: they arrive one AllGather earlier
            order = [0, 2, 4, 6, 1, 3, 5, 7]
            sel = order if half is None else order[4 * half:4 * half + 4]
            for i, hdt in enumerate(sel):
                nc.tensor.matmul(
                    op[:, 0:ES],
                    lhsT=lh_sl(qb, hdt, tt),
                    rhs=wsl("wo", hdt),
                    start=(half != 1 and i == 0),
                    stop=((half is None or half == 1) and i == len(sel) - 1),
                )
            if half == 0:
                return
            if qb not in osb_of:
                osb_of[qb] = (outp.tile([P, 4 * ES], BF16, tag="osb",
                                        name=f"osb{qb}"), [])
            osb, done = osb_of[qb]
            nc.vector.tensor_add(
                osb[:, tt * ES:(tt + 1) * ES], op[:, 0:ES], bias_sb[:])
            done.append(tt)
            if len(done) == 4:
                nc.sync.dma_start(
                    out[t0:t0 + TBLK, :].rearrange("(s p) e -> p s e", p=P),
                    osb[:].rearrange("p (s e) -> p s e", e=ES),
                )

        # --------- per-pair AllGather (pr = head pair 0/1 of this core) -----
        def emit_ag_cols(key, at, c0, cn):
            """Gather at[:, c0:c0+cn] across the 4 ranks of this batch;
            returns the wide SBUF tile with rank g at cols [g*cn, (g+1)*cn)."""
            ag_out = dramp.tile([GROUPS * P, cn], BF16, tag=f"agout{key}",
                                name=f"agout{key}")
            if with_collective:
                ag_in = dramp.tile([P, cn], BF16, tag=f"agin{key}",
                                   name=f"agin{key}")
                nc.sync.dma_start(ag_in[:], at[:, c0:c0 + cn])
                nc.gpsimd.collective_compute(
                    "AllGather",
                    mybir.AluOpType.bypass,
                    replica_groups=REPLICA_GROUPS,
                    ins=[ag_in[:].opt()],
                    outs=[ag_out[:].opt()],
                )
            else:
                # timing/sim variant: model the collective's local footprint
                # (own-contribution write; peer slots arrive over D2D, which
                # doesn't occupy the local DMA engines)
                nc.sync.dma_start(ag_out[0:P, :], at[:, c0:c0 + cn])
            lhw = lhp.tile([P, GROUPS * cn], BF16, tag="lh",
                           name=f"lh{key}")
            nc.sync.dma_start(
                lhw[:].rearrange("p (g t) -> p g t", g=GROUPS),
                ag_out[:].rearrange("(g p) t -> p g t", g=GROUPS),
            )
            return lhw

        def emit_ag(qb, pr, at):
            lh_of[qb][pr] = emit_ag_cols(f"{qb}_{pr}", at, 0, TBLK)

        # ---------------- emission schedule ----------------
        # startup DMAs in bus-priority order across two HWDGE queues; the
        # first Q/K DoubleRow matmul needs only wq8/x8(0) chunk-pair 0
        emit_w8_dma("wq", wq8, 0, NCT * ES)         # sync
        emit_x8_dma(0, 0, 2, eng=nc.scalar)         # scalar
        emit_w8_dma("wk", wk8, 0, NCT * ES)         # sync
        emit_x8_dma(0, 2, 6, eng=nc.scalar)         # scalar
        emit_x8_dma(1, eng=nc.sync)
        emit_w_dma("wv", wv, 0, NCT, eng=nc.scalar)
        nc.sync.dma_start(trbo_sb[:], trbo[:])
        emit_x_dma(0, eng=nc.scalar)                # bf16, for V
        emit_x_dma(1, eng=nc.sync)
        emit_w_dma("wo", wo, 0, NCT, eng=nc.scalar)

        def drive(gen, vfill, fillers, ns, vstart=None, pre_gen=None,
                  pre_at=(), at=None):
            """Drive a head pair's s-loop.  V fillers pop at ctrs
            vstart+1.. (early-mid loop, after their xT DMA but before their
            diagonal-step deadlines); generic fillers are spread evenly
            (Bresenham) over the remaining slots; pre_gen (the next pair) is
            advanced one step at each ctr in pre_at so its scores/exp keep
            ACT fed across the pair transition; `at` maps specific ctrs to
            extra closures (used to pin late work after a hook has fired)."""
            ctr = 0
            nf = len(fillers)
            done = 0
            vpops = 0
            if vstart is None:
                vstart = ns - len(vfill)
            den = max(1, ns - 2 - len(vfill))
            for _ in gen:
                ctr += 1
                if pre_gen is not None and ctr in pre_at:
                    next(pre_gen)
                if at and ctr in at:
                    for fn in at[ctr]:
                        fn()
                if vfill and ctr > vstart:
                    vfill.pop(0)()
                    vpops += 1
                else:
                    target = -(-(ctr - vpops) * nf // den)  # ceil
                    while done < target and fillers:
                        fillers.pop(0)()
                        done += 1

        lh_of = {qb: [None, None] for qb in range(NTB)}

        def new_at(qb, pr):
            return atp.tile([P, TBLK], BF16, tag="at", name=f"at{qb}_{pr}")

        def oproj_fillers(qb):
            # half-units (~0.45us) pack into sub-us pipeline holes
            units = []
            for tt in range(4):
                holder = {}
                units.append(lambda tt=tt, q=qb, h=holder:
                             emit_oproj_tt(q, tt, 0, h))
                units.append(lambda tt=tt, q=qb, h=holder:
                             emit_oproj_tt(q, tt, 1, h))
            return units

        def drain(lst):
            while lst:
                lst.pop(0)()

        def drain_gen(g):
            for _ in g:
                pass

        # ---- t-block 0: scores-first ----
        # PE order: p0 Q/K proj, all 8 scores/exp of both pairs (p1 proj and
        # block-1 p0 proj woven), then V + AV bursts with block-1 scores
        # trickling in via pre_at.  ACT runs exp back to back from ~5us
        # while xT/wv stream in for the V projections.
        qk0 = qk_units(0)
        qk1 = qk_units(1)
        qk0[0]()            # p0 Q
        qk0[1]()            # p0 K
        at00 = new_at(0, 0)
        at01 = new_at(0, 1)
        at10 = new_at(1, 0)
        g00 = emit_headpair(0, 0, at00, ahead=4)
        g01 = emit_headpair(0, 1, at01, ahead=4)
        g10 = emit_headpair(1, 0, at10, ahead=4)
        next(g00)           # s00_0
        next(g00)           # s00_1
        qk0[2]()            # p1 Q
        next(g00)           # s00_2
        qk0[3]()            # p1 K
        next(g00)           # s00_3
        for _ in range(4):
            next(g01)       # s01_0..3
        qk1[0]()            # block-1 p0 Q
        qk1[1]()            # block-1 p0 K
        v1 = v_chunks(1)
        next(g10)           # s10_0
        next(g10)           # s10_1  (ahead of the xT/wv DMA wall)
        v0 = v_chunks(0)
        v0.pop(0)()         # V(0) ahead of the first diagonal burst
        drive(g00, v0, [], 8, vstart=1, pre_gen=g10, pre_at=(3, 5))
        emit_ag(0, 0, at00)
        emit_x8_dma(2)
        emit_x_dma(2)
        drive(g01, v1[1:4], qk1[2:], 8, vstart=5, pre_gen=g10,
              pre_at=(1, 3, 5, 7))
        emit_ag(0, 1, at01)
        del v1[1:4]

        # ---- t-block 1 ----
        qk2 = qk_units(2)
        at11 = new_at(1, 1)
        g11 = emit_headpair(1, 1, at11, ahead=2)
        v1.pop(0)()         # V(4) ahead of g10's first diagonal burst
        drive(g10, v1, qk2[:2], 8, vstart=1, pre_gen=g11,
              pre_at=(1, 3, 5, 7))
        emit_ag(1, 0, at10)
        emit_x8_dma(3)
        emit_x_dma(3)
        opr0 = oproj_fillers(0)
        at20 = new_at(2, 0)
        g20 = emit_headpair(2, 0, at20, ahead=2)
        drive(g11, [], qk2[2:] + opr0[:4], 10, pre_gen=g20,
              pre_at=(3, 5, 7, 9))
        emit_ag(1, 1, at11)

        # ---- t-block 2 ----
        v2 = v_chunks(2)
        opr1 = oproj_fillers(1)
        at21 = new_at(2, 1)
        g21 = emit_headpair(2, 1, at21, ahead=2)
        drive(g20, v2, opr0[4:6], 14, vstart=2,
              pre_gen=g21, pre_at=(7, 9, 11, 13))
        emit_ag(2, 0, at20)
        qk3 = qk_units(3)
        at30 = new_at(3, 0)
        g30 = emit_headpair(3, 0, at30, ahead=2)
        drive(g21, [], qk3 + opr0[6:] + opr1[:2], 14, pre_gen=g30,
              pre_at=(7, 9, 11, 13))
        emit_ag(2, 1, at21)

        # ---- t-block 3 ----
        v3 = v_chunks(3)
        opr2 = oproj_fillers(2)
        at31 = new_at(3, 1)
        lh31 = {}
        # tt0+tt1 gather as one half; tt2 and tt3 each gather alone so the
        # final output projection only ever waits on the 128-col tile that
        # actually finished last
        agw = [2 * P, None, P, P]
        ag31out = {tt: dramp.tile([GROUPS * P, agw[tt]], BF16,
                                  tag=f"agout31{tt}", name=f"agout31{tt}")
                   for tt in (0, 2, 3)}
        ag31in = {tt: dramp.tile([P, agw[tt]], BF16, tag=f"agin31{tt}",
                                 name=f"agin31{tt}")
                  for tt in (0, 2, 3)}

        def hook31(tt):
            # stage each finished t-tile into its collective input as soon
            # as it exists; gathers ride the idle scalar queue
            key = 0 if tt < 2 else tt
            col = (tt % 2) * P if tt < 2 else 0
            src = at31[:, tt * P:(tt + 1) * P]
            stage = ag31in[key] if with_collective else ag31out[key]
            nc.scalar.dma_start(stage[0:P, col:col + P], src)
            if tt == 0:
                return
            if with_collective:
                nc.gpsimd.collective_compute(
                    "AllGather",
                    mybir.AluOpType.bypass,
                    replica_groups=REPLICA_GROUPS,
                    ins=[ag31in[key][:].opt()],
                    outs=[ag31out[key][:].opt()],
                )
            lhw = lhp.tile([P, GROUPS * agw[key]], BF16, tag="lh",
                           name=f"lh31_{key}")
            nc.scalar.dma_start(
                lhw[:].rearrange("p (g t) -> p g t", g=GROUPS),
                ag31out[key][:].rearrange("(g p) t -> p g t", g=GROUPS),
            )
            lh31[key] = lhw

        # final-block out-projection: one complete 8-matmul chain per t-tile
        # (pr-0 hd-tiles from AG(3,0), pr-1 from the half-AGs; in lh31
        # halves rank g sits at cols [g*256,+256), tt%2 picks the 128-col
        # t-tile).  The op accumulators live in the ps2 (scores) pool,
        # which is free once the last scores are emitted -- this keeps the
        # tail off the small-psum pool and its transpose-DMA WAR chains.
        # tt0/tt1 are injected into g31's loop right after the first
        # half-AllGather fires (via drive's `at`); tt2/tt3 follow the loop.
        tz = (NTB - 1) * TBLK
        osbz = outp.tile([P, 4 * ES], BF16, tag="osb", name="osbz")
        opz = [None]

        def emit_tz_tt(tt):
            if opz[0] is None:
                opz[0] = ps2.tile([P, 2 * TBLK], F32, tag="st", name="opz")
            op = opz[0][:, tt * ES:(tt + 1) * ES]
            key = 0 if tt < 2 else tt
            half = lh31[key]
            w = 2 * P if tt < 2 else P
            col = (tt % 2) * P if tt < 2 else 0
            for i, hdt in enumerate((0, 2, 4, 6)):
                nc.tensor.matmul(
                    op, lhsT=lh_sl(3, hdt, tt), rhs=wsl("wo", hdt),
                    start=(i == 0), stop=(i == 3),
                )
            for j, hdt in enumerate((1, 3, 5, 7)):
                g_ = (hdt - 1) // 2
                nc.tensor.matmul(
                    op,
                    lhsT=half[:, g_ * w + col:g_ * w + col + P],
                    rhs=wsl("wo", hdt),
                    start=False, stop=(j == 3),
                )
            nc.vector.tensor_add(
                osbz[:, tt * ES:(tt + 1) * ES], op, bias_sb[:])
            nc.sync.dma_start(
                out[tz + tt * P:tz + (tt + 1) * P, :],
                osbz[:, tt * ES:(tt + 1) * ES],
            )

        # last pair: transposes NOT deferred, so the per-t-tile staging DMAs
        # fire as early as possible
        g31 = emit_headpair(3, 1, at31, hook_tt=hook31, defer_tp=False,
                            ahead=2)
        drive(g30, v3, opr1[2:], 18, vstart=6, pre_gen=g31,
              pre_at=(11, 13, 15, 17))
        emit_ag(3, 0, at30)
        drive(g31, [], opr2, 14,
              at={15: [lambda: emit_tz_tt(0)], 17: [lambda: emit_tz_tt(1)]})
        drain(opr2)
        emit_tz_tt(2)
        emit_tz_tt(3)

    nc.compile()
    return nc


_NC_CACHE = {}


def _get_nc(with_collective=True):
    key = with_collective
    if key not in _NC_CACHE:
        _NC_CACHE[key] = build_nc(with_collective)
    return _NC_CACHE[key]


def make_in_maps(x, Wq, Wk, Wv, Wo, bo):
    import ml_dtypes
    bf16 = ml_dtypes.bfloat16
    f8 = ml_dtypes.float8_e4m3
    trbo = np.concatenate(
        [np.triu(np.ones((P, P), dtype=np.float32)),
         np.eye(P, dtype=np.float32),
         np.zeros((P, ES), dtype=np.float32)], axis=1)
    in_maps = []
    for c in range(N_CORES):
        b, g = c // GROUPS, c % GROUPS
        hs_ = slice(g * HPG, (g + 1) * HPG)
        tb = trbo.copy()
        tb[:, 2 * P:] = bo[g * ES:(g + 1) * ES][None, :]

        def shuffle8(W):
            # [C, HD] -> SBUF image [P, NCT*HD] (chunk ci at cols ci*HD)
            w = (W[hs_].transpose(1, 0, 2).reshape(C, HD) * WSC)
            return np.ascontiguousarray(
                w.reshape(NCT, P, HD).transpose(1, 0, 2).reshape(
                    P, NCT * HD)).astype(f8)

        in_maps.append({
            "xT": np.ascontiguousarray(x[b].T).astype(bf16),
            "x8": np.ascontiguousarray(x[b].T * XSC).astype(f8),
            "wq": shuffle8(Wq),
            "wk": shuffle8(Wk),
            "wv": np.ascontiguousarray(
                Wv[hs_].transpose(1, 0, 2).reshape(C, HD)).astype(bf16),
            "wo": np.ascontiguousarray(
                Wo[:, g * ES:(g + 1) * ES]).astype(bf16),
            "trbo": np.ascontiguousarray(tb).astype(bf16),
        })
    return in_maps


def kernel(x, Wq, Wk, Wv, Wo, bo):
    x = np.asarray(x, dtype=np.float32)
    Wq = np.asarray(Wq, dtype=np.float32)
    Wk = np.asarray(Wk, dtype=np.float32)
    Wv = np.asarray(Wv, dtype=np.float32)
    Wo = np.asarray(Wo, dtype=np.float32)
    bo = np.asarray(bo, dtype=np.float32)

    nc = _get_nc(with_collective=True)
    in_maps = make_in_maps(x, Wq, Wk, Wv, Wo, bo)
    res = run_bass_kernel_spmd(nc, in_maps, core_ids=list(range(N_CORES)))

    out = np.empty((B, T, E), dtype=np.float32)
    for c in range(N_CORES):
        b, g = c // GROUPS, c % GROUPS
        out[b, :, g * ES:(g + 1) * ES] = np.asarray(
            res.results[c]["out"], dtype=np.float32)
    return out


# revision 28
# speedup vs baseline: 1.0039x; 1.0031x over previous
"""Multi-head causal self-attention on 8 TRN2 NeuronCores (bf16 + fp8 QK).

Problem (nn_MultiHeadAttention): B=2, T=2048, C=1024, H=16 heads, hs=64.
  q,k,v = per-head projections of x; causal softmax(q k^T / 8) v;
  concat heads; out = att @ Wo + bo.

Sharding: core c in 0..7 -> (batch b = c//4, head-group g = c%4, 4 heads each).
Each core computes Q/K/V + causal attention for its 4 heads on its batch,
normalized attention outputs are AllGathered across the 4 cores of the same
batch (replica groups [0-3], [4-7]), then each core computes a disjoint
256-column slice of the output projection (column-parallel Wo) + bias slice.
Host does a pure concat of the 8 disjoint output slices.

Q/K projections run in fp8e4 DoubleRow mode: W (x64) and x (x4) are quantized
host-side, the PE contracts TWO 128-row c-chunks per pass at 0.5 cycles/row
(4x the bf16 rate), and the combined 2^16 scale is folded into the softmax
exp scale (0.125 / 65536).  Quantization error enters only through the
softmax logits, where it attenuates to ~1% of the output.  V, the attention
matmuls, and the output projection stay bf16 (their error hits the output
directly); PSUM accumulation is fp32 throughout.

Attention layout: scores stay transposed (S^T [s, t], lhsT=K^T, rhs=Q^T),
exp runs on ACT into bf16 P^T tiles, and AV is computed non-transposed:
out[t_p, d_f] += P^T[s, t-tile].T @ V[s, d] with full 128 output partitions.
Softmax denominators accumulate via F=1 ones-column matmuls into a shared
PSUM tile; normalization is a per-partition reciprocal + tensor_scalar
multiply on DVE.  The normalized A [t, d] tiles are transposed back to
A^T [d, t] on the PE for the AllGather + output projection.

Scheduling: the kernel is paced by ACT (exp) once PE work is cut by fp8, so
block 0 runs "scores-first": both head pairs' scores/exp for all four
s-tiles are emitted before any V-projection or AV work, keeping ACT fed
from ~3us while the x/W bf16 DMAs land.  Later blocks use the woven s-loop
schedule (scores si+1 emitted before AV si; per-(head,t-tile) AV bursts at
the diagonal step; projections/output-projections as pipeline fillers, one
block earlier than their consumers).
"""

import numpy as np
from contextlib import ExitStack

import concourse.bass as bass
import concourse.mybir as mybir
import concourse.tile as tile
from concourse import bacc
from concourse.bass_utils import run_bass_kernel_spmd

F32 = mybir.dt.float32
BF16 = mybir.dt.bfloat16
FP8 = mybir.dt.float8e4
EXP = mybir.ActivationFunctionType.Exp
DR = mybir.MatmulPerfMode.DoubleRow

N_CORES = 8
B = 2
T = 2048
C = 1024
NH = 16
HS = 64
E = 1024
GROUPS = 4          # head groups (tensor-parallel ranks per batch)
HPG = NH // GROUPS  # 4 heads per core
ES = E // GROUPS    # 256 output columns per core
HD = HPG * HS       # 256 local attention-output rows

P = 128             # partition tile
TBLK = 512          # t-block (matmul moving dim)
NTB = T // TBLK     # 4
NCT = C // P        # 8 contraction tiles for projections
NST = T // P        # 16 key tiles

WSC = 64.0          # host-side fp8 scale on Wq/Wk
XSC = 4.0           # host-side fp8 scale on x
EXPSC = 0.125 / (WSC * XSC) ** 2   # exp(EXPSC * S_scaled) = exp(S / sqrt(hs))

REPLICA_GROUPS = [[0, 1, 2, 3], [4, 5, 6, 7]]


def build_nc(with_collective=True):
    """Build + compile the per-core SPMD program. Same program on all cores."""
    nc = bacc.Bacc(
        "TRN2", target_bir_lowering=False, debug=False, num_devices=N_CORES
    )

    xT = nc.dram_tensor("xT", [C, T], BF16, kind="ExternalInput").ap()
    x8T = nc.dram_tensor("x8", [C, T], FP8, kind="ExternalInput").ap()
    # fp8 weights arrive pre-shuffled to the SBUF image [p, (c d)] so the DMA
    # moves one contiguous 2KB run per partition
    wq8 = nc.dram_tensor("wq", [P, NCT * ES], FP8, kind="ExternalInput").ap()
    wk8 = nc.dram_tensor("wk", [P, NCT * ES], FP8, kind="ExternalInput").ap()
    wv = nc.dram_tensor("wv", [C, HD], BF16, kind="ExternalInput").ap()
    wo = nc.dram_tensor("wo", [E, ES], BF16, kind="ExternalInput").ap()
    # tri (cols 0:P), identity (cols P:2P), broadcast bias (cols 2P:2P+ES)
    trbo = nc.dram_tensor("trbo", [P, 2 * P + ES], BF16,
                          kind="ExternalInput").ap()
    out = nc.dram_tensor("out", [T, ES], BF16, kind="ExternalOutput").ap()

    with tile.TileContext(nc) as tc, ExitStack() as ctx:
        wp = ctx.enter_context(tc.tile_pool(name="wp", bufs=1))
        xp = ctx.enter_context(tc.tile_pool(name="xp", bufs=3))
        x8p = ctx.enter_context(tc.tile_pool(name="x8p", bufs=3))
        qkp = ctx.enter_context(tc.tile_pool(name="qkp", bufs=1))
        vp = ctx.enter_context(tc.tile_pool(name="vp", bufs=1))
        ptp = ctx.enter_context(tc.tile_pool(name="ptp", bufs=22))
        anp = ctx.enter_context(tc.tile_pool(name="anp", bufs=4))
        atp = ctx.enter_context(tc.tile_pool(name="atp", bufs=3))
        smp = ctx.enter_context(tc.tile_pool(name="smp", bufs=6))
        outp = ctx.enter_context(tc.tile_pool(name="outp", bufs=4))
        lhp = ctx.enter_context(tc.tile_pool(name="lhp", bufs=10))
        # PSUM: 8 banks.  st [128,1024] x2 bufs = 4 banks, attps 1, denps 1,
        # small (qkv-proj / oproj / transpose) 2.
        ps2 = ctx.enter_context(tc.tile_pool(name="ps2", bufs=2, space="PSUM"))
        psA = ctx.enter_context(tc.tile_pool(name="psA", bufs=1, space="PSUM"))
        psD = ctx.enter_context(tc.tile_pool(name="psD", bufs=1, space="PSUM"))
        psC = ctx.enter_context(tc.tile_pool(name="psC", bufs=2, space="PSUM"))
        dramp = ctx.enter_context(tc.tile_pool(name="dramp", bufs=1, space="DRAM"))

        # ---- small constants ----
        trbo_sb = wp.tile([P, 2 * P + ES], BF16, tag="trbo")
        tri_sb = trbo_sb[:, 0:P]
        id_sb = trbo_sb[:, P:2 * P]
        bias_sb = trbo_sb[:, 2 * P:2 * P + ES]
        onescol = wp.tile([P, 1], BF16, tag="onescol")
        nc.vector.memset(onescol[:], 1.0)

        # weights: one wide tile per tensor, chunk ci at cols [ci*ES, ...)
        w_sb = {
            "wq": wp.tile([P, NCT * ES], FP8, tag="w_wq", name="w_wq"),
            "wk": wp.tile([P, NCT * ES], FP8, tag="w_wk", name="w_wk"),
            "wv": wp.tile([P, NCT * ES], BF16, tag="w_wv", name="w_wv"),
            "wo": wp.tile([P, NCT * ES], BF16, tag="w_wo", name="w_wo"),
        }

        def wsl(name, ci):
            return w_sb[name][:, ci * ES:(ci + 1) * ES]

        def w8sl(name, i, pr):
            # chunk-pair i of the fp8 weights as a DoubleRow lhsT
            # [p, 2 k-tiles, 128 out-rows] for head pair pr
            return w_sb[name][:, 2 * i * ES:(2 * i + 2) * ES].rearrange(
                "p (c m) -> p c m", m=ES)[:, :, pr * P:(pr + 1) * P]

        def emit_w_dma(name, dram, ci0, nch, eng=None):
            (eng or nc.sync).dma_start(
                w_sb[name][:, ci0 * ES:(ci0 + nch) * ES].rearrange(
                    "p (c d) -> p c d", d=ES),
                dram.rearrange("(c p) d -> p c d", p=P)[:, ci0:ci0 + nch, :],
            )

        def emit_w8_dma(name, dram, c0, c1, eng=None):
            (eng or nc.sync).dma_start(w_sb[name][:, c0:c1], dram[:, c0:c1])

        # denominators: slice (pr, head-in-pair, tt) -> one fp32 column
        denps = psD.tile([P, 16], F32, tag="denps")

        # x^T per t-block (bf16 for V): chunk ci at cols [ci*TBLK, ...)
        xw_of = {}

        def xsl(tb, ci):
            return xw_of[tb][:, ci * TBLK:(ci + 1) * TBLK]

        # x fp8 per t-block (for Q/K): chunk-pair i as DoubleRow rhs
        x8_of = {}

        def x8sl(tb, i):
            return x8_of[tb][:, 2 * i * TBLK:(2 * i + 2) * TBLK].rearrange(
                "p (c t) -> p c t", t=TBLK)

        # merged Q^T/K^T per head pair: col = tb*1024 + qk*512 + t_local
        # (pair p holds heads 2p (rows 0-63) and 2p+1 (rows 64-127))
        qkt = [qkp.tile([P, 2 * T], BF16, tag=f"qk{p_}", name=f"qk{p_}")
               for p_ in range(2)]

        def qt_slice(pr, r0, rn, t0, tn):
            tb, tl = t0 // TBLK, t0 % TBLK
            base = tb * 1024 + tl
            return qkt[pr][r0:r0 + rn, base:base + tn]

        def kt_slice(pr, r0, rn, s0, sn):
            tb, sl = s0 // TBLK, s0 % TBLK
            base = tb * 1024 + TBLK + sl
            return qkt[pr][r0:r0 + rn, base:base + sn]

        v_sb = [vp.tile([P, HPG * HS], BF16, tag=f"v{st}", name=f"v{st}")
                for st in range(NST)]

        # ---------------- stage-1 pieces ----------------
        def emit_x_dma(tb, eng=None):
            ts_ = tb * TBLK
            xw_of[tb] = xp.tile([P, NCT * TBLK], BF16, tag="xw",
                                name=f"xw{tb}")
            (eng or nc.sync).dma_start(
                xw_of[tb][:].rearrange("p (c t) -> p c t", t=TBLK),
                xT.rearrange("(c p) t -> p c t", p=P)[:, :, ts_:ts_ + TBLK],
            )

        def emit_x8_dma(tb, ci0=0, nch=NCT, eng=None):
            ts_ = tb * TBLK
            if tb not in x8_of:
                x8_of[tb] = x8p.tile([P, NCT * TBLK], FP8, tag="x8",
                                     name=f"x8_{tb}")
            (eng or nc.sync).dma_start(
                x8_of[tb][:, ci0 * TBLK:(ci0 + nch) * TBLK].rearrange(
                    "p (c t) -> p c t", t=TBLK),
                x8T.rearrange("(c p) t -> p c t", p=P)[
                    :, ci0:ci0 + nch, ts_:ts_ + TBLK],
            )

        def emit_qk_proj(tb, pr, which):
            # fp8 DoubleRow: 4 chunk-pair matmuls cover all 8 c-chunks at
            # 0.5 cycles/row -> ~0.43us of PE per (tb, pr, which)
            wn = "wq" if which == 0 else "wk"
            ps = psC.tile([P, TBLK], F32, tag="small",
                          name=f"qkps{tb}_{pr}_{which}")
            for i in range(4):
                nc.tensor.matmul(
                    ps[:], lhsT=w8sl(wn, i, pr), rhs=x8sl(tb, i),
                    start=(i == 0), stop=(i == 3), perf_mode=DR,
                )
            base = tb * 1024 + which * TBLK
            nc.vector.tensor_copy(qkt[pr][:, base:base + TBLK], ps[:])

        def qk_units(tb):
            # 4 units per t-block: (pr0,Q), (pr0,K), (pr1,Q), (pr1,K)
            return [lambda pr=pr, w=w: emit_qk_proj(tb, pr, w)
                    for pr in range(2) for w in range(2)]

        def emit_v_proj(st):
            tb, sl = st // 4, (st % 4) * P
            vps = psC.tile([P, TBLK], F32, tag="small", name=f"vps{st}")
            for ci in range(NCT):
                nc.tensor.matmul(
                    vps[:, 0:HD],
                    lhsT=xsl(tb, ci)[:, sl:sl + P],
                    rhs=wsl("wv", ci),
                    start=(ci == 0), stop=(ci == NCT - 1),
                )
            nc.vector.tensor_copy(v_sb[st][:], vps[:, 0:HD])

        def v_chunks(tb):
            return [lambda st=st: emit_v_proj(st)
                    for st in range(4 * tb, 4 * tb + 4)]

        # ------- stage-2: one head PAIR of one t-block ----------------------
        def emit_headpair(qb, pr, at, hook_tt=None, defer_tp=True, ahead=1):
            """s-loop over key tiles; both heads of the pair per step.  AV is
            non-transposed: attps[t_p, (par,tt) 64-col slice] with per-slice
            fp32 denominator columns in denps.  Each (par,tt) slice finishes
            at its diagonal s-step -> finalize (normalize + transpose into
            `at`) is woven in right there.

            `ahead` scores/exp steps are emitted before the first AV burst
            (ahead=1 is the classic software pipeline: scores(si+1) before
            AV(si); block 0 uses ahead=4 to front-run every exp past the
            V-projection DMA wall).  One yield per scores step, then one
            yield per s-tile of the burst phase.  hook_tt, if given, is
            called after finalize(tt)."""
            t0 = qb * TBLK
            ns = 4 * (qb + 1)
            attps = psA.tile([P, 4 * P], F32, tag="attps",
                             name=f"attps{qb}_{pr}")

            def scores_exp(si):
                diag = si * P >= t0
                ka = si * P - t0 if diag else 0
                stp = ps2.tile([P, 2 * TBLK], F32, tag="st",
                               name=f"st{qb}_{pr}_{si}")
                for par in range(2):
                    r0 = par * HS
                    nc.tensor.matmul(
                        stp[:, par * TBLK + ka:(par + 1) * TBLK],
                        lhsT=kt_slice(pr, r0, HS, si * P, P),
                        rhs=qt_slice(pr, r0, HS, t0 + ka, TBLK - ka),
                        start=True, stop=True,
                    )
                pt = ptp.tile([P, 2 * TBLK], BF16, tag="pt",
                              name=f"pt{qb}_{pr}_{si}")
                if ka > 0:
                    s3 = stp[:].rearrange("p (r c) -> p r c", r=2)[:, :, ka:TBLK]
                    p3 = pt[:].rearrange("p (r c) -> p r c", r=2)[:, :, ka:TBLK]
                    nc.scalar.activation(p3, s3, EXP, scale=EXPSC)
                else:
                    nc.scalar.activation(pt[:], stp[:], EXP, scale=EXPSC)
                return pt

            pending = []  # deferred transpose closures (one s-step later)

            def emit_tp(tt, an):
                tp = psC.tile([P, TBLK], BF16, tag="small",
                              name=f"tp{qb}_{pr}_{tt}",
                              padded_shape=[P, 2 * TBLK])
                nc.tensor.transpose(tp[:, 0:P], an[:], id_sb[:])
                nc.vector.tensor_copy(at[:, tt * P:(tt + 1) * P],
                                      tp[:, 0:P])
                if hook_tt is not None:
                    hook_tt(tt)

            def burst(si, tt, par, lo=0, hi=None, close=True):
                # den group first: its F=1 matmuls finish almost instantly,
                # so the DVE reciprocal overlaps the AV burst
                h = 2 * pr + par
                sl = (par * 4 + tt) * HS
                dc = pr * 8 + par * 4 + tt
                hi_ = si if hi is None else hi
                for sj in range(lo, hi_ + 1):
                    lhs = pts[sj][
                        :, par * TBLK + tt * P:par * TBLK + (tt + 1) * P]
                    nc.tensor.matmul(
                        denps[:, dc:dc + 1],
                        lhsT=lhs, rhs=onescol[:],
                        start=(sj == 0), stop=(close and sj == hi_),
                    )
                for sj in range(lo, hi_ + 1):
                    lhs = pts[sj][
                        :, par * TBLK + tt * P:par * TBLK + (tt + 1) * P]
                    nc.tensor.matmul(
                        attps[:, sl:sl + HS],
                        lhsT=lhs,
                        rhs=v_sb[sj][:, h * HS:(h + 1) * HS],
                        start=(sj == 0), stop=(close and sj == hi_),
                    )

            def norm(tt, par, an):
                # normalize on DVE; par0's chain overlaps par1's PE burst
                sl = (par * 4 + tt) * HS
                dc = pr * 8 + par * 4 + tt
                rc = smp.tile([P, 1], F32, tag="rc")
                nc.vector.reciprocal(rc[:], denps[:, dc:dc + 1])
                nc.vector.tensor_scalar_mul(
                    an[:, par * HS:(par + 1) * HS],
                    attps[:, sl:sl + HS], rc[:])

            # PSUM accumulation groups must not interleave within a bank on
            # real hardware, so AV for each (head, t-tile, par) is ONE
            # contiguous matmul burst over all its s-tiles, emitted at that
            # t-tile's diagonal s-step; every pt tile of the pair stays
            # resident until its last burst.  attps and denps are separate
            # banks, so their per-(head,t-tile) groups may alternate.  The
            # two par bursts straddle a yield so a filler or the next scores
            # emission lands between them (keeps the exp cadence smooth).
            pts = []
            for i in range(ahead):
                pts.append(scores_exp(i))
                yield
            for si in range(ns):
                if si + ahead < ns:
                    pts.append(scores_exp(si + ahead))
                diag = si * P >= t0
                if diag:
                    tt = (si * P - t0) // P
                    for par in range(2):
                        c0 = par * TBLK + tt * P
                        nc.vector.tensor_mul(
                            pts[si][:, c0:c0 + P], pts[si][:, c0:c0 + P],
                            tri_sb[:])
                    burst(si, tt, 0)
                yield
                while pending:
                    pending.pop(0)()
                if not diag:
                    continue
                burst(si, tt, 1)
                an = anp.tile([P, P], BF16, tag="an",
                              name=f"an{qb}_{pr}_{tt}")
                for par in range(2):
                    norm(tt, par, an)
                if defer_tp:
                    pending.append(lambda tt=tt, an=an: emit_tp(tt, an))
                else:
                    emit_tp(tt, an)
                yield
            while pending:
                pending.pop(0)()

        # ---------------- stage-3: one t-tile of one t-block ---------------
        # lh_of[qb] = [lhw_pr0, lhw_pr1]: wide AllGathered A^T tiles, rank g
        # at cols [g*TBLK, (g+1)*TBLK).  hdt (wo row-chunk) = 2g + pr.
        osb_of = {}

        def lh_sl(qb, hdt, tt):
            return lh_of[qb][hdt % 2][:, (hdt // 2) * TBLK + tt * P:
                                      (hdt // 2) * TBLK + (tt + 1) * P]

        def emit_oproj_tt(qb, tt, half=None, holder=None):
            t0 = qb * TBLK
            if half == 1:
                op = holder["ps"]
            else:
                op = psC.tile([P, TBLK], F32, tag="small", name=f"op{qb}_{tt}")
                if holder is not None:
                    holder["ps"] = op
            # pr0 tiles (even hdt) first: they arrive one AllGather earlier
            order = [0, 2, 4, 6, 1, 3, 5, 7]
            sel = order if half is None else order[4 * half:4 * half + 4]
            for i, hdt in enumerate(sel):
                nc.tensor.matmul(
                    op[:, 0:ES],
                    lhsT=lh_sl(qb, hdt, tt),
                    rhs=wsl("wo", hdt),
                    start=(half != 1 and i == 0),
                    stop=((half is None or half == 1) and i == len(sel) - 1),
                )
            if half == 0:
                return
            if qb not in osb_of:
                osb_of[qb] = (outp.tile([P, 4 * ES], BF16, tag="osb",
                                        name=f"osb{qb}"), [])
            osb, done = osb_of[qb]
            nc.vector.tensor_add(
                osb[:, tt * ES:(tt + 1) * ES], op[:, 0:ES], bias_sb[:])
            done.append(tt)
            if len(done) == 4:
                nc.sync.dma_start(
                    out[t0:t0 + TBLK, :].rearrange("(s p) e -> p s e", p=P),
                    osb[:].rearrange("p (s e) -> p s e", e=ES),
                )

        # --------- per-pair AllGather (pr = head pair 0/1 of this core) -----
        def emit_ag_cols(key, at, c0, cn):
            """Gather at[:, c0:c0+cn] across the 4 ranks of this batch;
            returns the wide SBUF tile with rank g at cols [g*cn, (g+1)*cn)."""
            ag_out = dramp.tile([GROUPS * P, cn], BF16, tag=f"agout{key}",
                                name=f"agout{key}")
            if with_collective:
                ag_in = dramp.tile([P, cn], BF16, tag=f"agin{key}",
                                   name=f"agin{key}")
                nc.sync.dma_start(ag_in[:], at[:, c0:c0 + cn])
                nc.gpsimd.collective_compute(
                    "AllGather",
                    mybir.AluOpType.bypass,
                    replica_groups=REPLICA_GROUPS,
                    ins=[ag_in[:].opt()],
                    outs=[ag_out[:].opt()],
                )
            else:
                # timing/sim variant: model the collective's local footprint
                # (own-contribution write; peer slots arrive over D2D, which
                # doesn't occupy the local DMA engines)
                nc.sync.dma_start(ag_out[0:P, :], at[:, c0:c0 + cn])
            lhw = lhp.tile([P, GROUPS * cn], BF16, tag="lh",
                           name=f"lh{key}")
            nc.sync.dma_start(
                lhw[:].rearrange("p (g t) -> p g t", g=GROUPS),
                ag_out[:].rearrange("(g p) t -> p g t", g=GROUPS),
            )
            return lhw

        def emit_ag(qb, pr, at):
            lh_of[qb][pr] = emit_ag_cols(f"{qb}_{pr}", at, 0, TBLK)

        # ---------------- emission schedule ----------------
        # startup DMAs in bus-priority order across two HWDGE queues; the
        # first Q/K DoubleRow matmul needs only wq8/x8(0) chunk-pair 0
        emit_w8_dma("wq", wq8, 0, NCT * ES)         # sync
        emit_x8_dma(0, 0, 2, eng=nc.scalar)         # scalar
        emit_w8_dma("wk", wk8, 0, NCT * ES)         # sync
        emit_x8_dma(0, 2, 6, eng=nc.scalar)         # scalar
        emit_x8_dma(1, eng=nc.sync)
        emit_w_dma("wv", wv, 0, NCT, eng=nc.scalar)
        nc.sync.dma_start(trbo_sb[:], trbo[:])
        emit_x_dma(0, eng=nc.scalar)                # bf16, for V
        emit_x_dma(1, eng=nc.sync)
        emit_w_dma("wo", wo, 0, NCT, eng=nc.scalar)

        def drive(gen, vfill, fillers, ns, vstart=None, pre_gen=None,
                  pre_at=(), at=None):
            """Drive a head pair's s-loop.  V fillers pop at ctrs
            vstart+1.. (early-mid loop, after their xT DMA but before their
            diagonal-step deadlines); generic fillers are spread evenly
            (Bresenham) over the remaining slots; pre_gen (the next pair) is
            advanced one step at each ctr in pre_at so its scores/exp keep
            ACT fed across the pair transition; `at` maps specific ctrs to
            extra closures (used to pin late work after a hook has fired)."""
            ctr = 0
            nf = len(fillers)
            done = 0
            vpops = 0
            if vstart is None:
                vstart = ns - len(vfill)
            den = max(1, ns - 2 - len(vfill))
            for _ in gen:
                ctr += 1
                if pre_gen is not None and ctr in pre_at:
                    next(pre_gen)
                if at and ctr in at:
                    for fn in at[ctr]:
                        fn()
                if vfill and ctr > vstart:
                    vfill.pop(0)()
                    vpops += 1
                else:
                    target = -(-(ctr - vpops) * nf // den)  # ceil
                    while done < target and fillers:
                        fillers.pop(0)()
                        done += 1

        lh_of = {qb: [None, None] for qb in range(NTB)}

        def new_at(qb, pr):
            return atp.tile([P, TBLK], BF16, tag="at", name=f"at{qb}_{pr}")

        def oproj_fillers(qb):
            # half-units (~0.45us) pack into sub-us pipeline holes
            units = []
            for tt in range(4):
                holder = {}
                units.append(lambda tt=tt, q=qb, h=holder:
                             emit_oproj_tt(q, tt, 0, h))
                units.append(lambda tt=tt, q=qb, h=holder:
                             emit_oproj_tt(q, tt, 1, h))
            return units

        def drain(lst):
            while lst:
                lst.pop(0)()

        def drain_gen(g):
            for _ in g:
                pass

        # ---- t-block 0: scores-first ----
        # PE order: p0 Q/K proj, all 8 scores/exp of both pairs (p1 proj and
        # block-1 p0 proj woven), then V + AV bursts with block-1 scores
        # trickling in via pre_at.  ACT runs exp back to back from ~5us
        # while xT/wv stream in for the V projections.
        qk0 = qk_units(0)
        qk1 = qk_units(1)
        qk0[0]()            # p0 Q
        qk0[1]()            # p0 K
        at00 = new_at(0, 0)
        at01 = new_at(0, 1)
        at10 = new_at(1, 0)
        g00 = emit_headpair(0, 0, at00, ahead=4)
        g01 = emit_headpair(0, 1, at01, ahead=4)
        g10 = emit_headpair(1, 0, at10, ahead=4)
        next(g00)           # s00_0
        next(g00)           # s00_1
        qk0[2]()            # p1 Q
        next(g00)           # s00_2
        qk0[3]()            # p1 K
        next(g00)           # s00_3
        for _ in range(4):
            next(g01)       # s01_0..3
        qk1[0]()            # block-1 p0 Q
        qk1[1]()            # block-1 p0 K
        v1 = v_chunks(1)
        next(g10)           # s10_0
        next(g10)           # s10_1  (ahead of the xT/wv DMA wall)
        v0 = v_chunks(0)
        v0.pop(0)()         # V(0) ahead of the first diagonal burst
        drive(g00, v0, [], 8, vstart=1, pre_gen=g10, pre_at=(3, 5))
        emit_ag(0, 0, at00)
        emit_x8_dma(2)
        emit_x_dma(2)
        drive(g01, v1[1:4], qk1[2:], 8, vstart=5, pre_gen=g10,
              pre_at=(1, 3, 5, 7))
        emit_ag(0, 1, at01)
        del v1[1:4]

        # ---- t-block 1 ----
        qk2 = qk_units(2)
        at11 = new_at(1, 1)
        g11 = emit_headpair(1, 1, at11, ahead=2)
        v1.pop(0)()         # V(4) ahead of g10's first diagonal burst
        drive(g10, v1, qk2[:2], 8, vstart=1, pre_gen=g11,
              pre_at=(1, 3, 5, 7))
        emit_ag(1, 0, at10)
        emit_x8_dma(3)
        emit_x_dma(3)
        opr0 = oproj_fillers(0)
        at20 = new_at(2, 0)
        g20 = emit_headpair(2, 0, at20, ahead=2)
        drive(g11, [], qk2[2:] + opr0[:4], 10, pre_gen=g20,
              pre_at=(3, 5, 7, 9))
        emit_ag(1, 1, at11)

        # ---- t-block 2 ----
        v2 = v_chunks(2)
        opr1 = oproj_fillers(1)
        at21 = new_at(2, 1)
        g21 = emit_headpair(2, 1, at21, ahead=2)
        drive(g20, v2, opr0[4:6], 14, vstart=2,
              pre_gen=g21, pre_at=(7, 9, 11, 13))
        emit_ag(2, 0, at20)
        qk3 = qk_units(3)
        at30 = new_at(3, 0)
        g30 = emit_headpair(3, 0, at30, ahead=2)
        drive(g21, [], qk3 + opr0[6:] + opr1[:2], 14, pre_gen=g30,
              pre_at=(7, 9, 11, 13))
        emit_ag(2, 1, at21)

        # ---- t-block 3 ----
        v3 = v_chunks(3)
        opr2 = oproj_fillers(2)
        at31 = new_at(3, 1)
        lh31 = {}
        # tt0+tt1 gather as one half; tt2 and tt3 each gather alone so the
        # final output projection only ever waits on the 128-col tile that
        # actually finished last
        agw = [2 * P, None, P, P]
        ag31out = {tt: dramp.tile([GROUPS * P, agw[tt]], BF16,
                                  tag=f"agout31{tt}", name=f"agout31{tt}")
                   for tt in (0, 2, 3)}
        ag31in = {tt: dramp.tile([P, agw[tt]], BF16, tag=f"agin31{tt}",
                                 name=f"agin31{tt}")
                  for tt in (0, 2, 3)}

        def hook31(tt):
            # stage each finished t-tile into its collective input as soon
            # as it exists; gathers ride the idle scalar queue
            key = 0 if tt < 2 else tt
            col = (tt % 2) * P if tt < 2 else 0
            src = at31[:, tt * P:(tt + 1) * P]
            stage = ag31in[key] if with_collective else ag31out[key]
            nc.sync.dma_start(stage[0:P, col:col + P], src)
            if tt == 0:
                return
            if with_collective:
                nc.gpsimd.collective_compute(
                    "AllGather",
                    mybir.AluOpType.bypass,
                    replica_groups=REPLICA_GROUPS,
                    ins=[ag31in[key][:].opt()],
                    outs=[ag31out[key][:].opt()],
                )
            lhw = lhp.tile([P, GROUPS * agw[key]], BF16, tag="lh",
                           name=f"lh31_{key}")
            nc.sync.dma_start(
                lhw[:].rearrange("p (g t) -> p g t", g=GROUPS),
                ag31out[key][:].rearrange("(g p) t -> p g t", g=GROUPS),
            )
            lh31[key] = lhw

        # final-block out-projection: one complete 8-matmul chain per t-tile
        # (pr-0 hd-tiles from AG(3,0), pr-1 from the half-AGs; in lh31
        # halves rank g sits at cols [g*256,+256), tt%2 picks the 128-col
        # t-tile).  The op accumulators live in the ps2 (scores) pool,
        # which is free once the last scores are emitted -- this keeps the
        # tail off the small-psum pool and its transpose-DMA WAR chains.
        # tt0/tt1 are injected into g31's loop right after the first
        # half-AllGather fires (via drive's `at`); tt2/tt3 follow the loop.
        tz = (NTB - 1) * TBLK
        osbz = outp.tile([P, 4 * ES], BF16, tag="osb", name="osbz")
        opz = [None]

        def emit_tz_tt(tt):
            if opz[0] is None:
                opz[0] = ps2.tile([P, 2 * TBLK], F32, tag="st", name="opz")
            op = opz[0][:, tt * ES:(tt + 1) * ES]
            key = 0 if tt < 2 else tt
            half = lh31[key]
            w = 2 * P if tt < 2 else P
            col = (tt % 2) * P if tt < 2 else 0
            for i, hdt in enumerate((0, 2, 4, 6)):
                nc.tensor.matmul(
                    op, lhsT=lh_sl(3, hdt, tt), rhs=wsl("wo", hdt),
                    start=(i == 0), stop=(i == 3),
                )
            for j, hdt in enumerate((1, 3, 5, 7)):
                g_ = (hdt - 1) // 2
                nc.tensor.matmul(
                    op,
                    lhsT=half[:, g_ * w + col:g_ * w + col + P],
                    rhs=wsl("wo", hdt),
                    start=False, stop=(j == 3),
                )
            nc.vector.tensor_add(
                osbz[:, tt * ES:(tt + 1) * ES], op, bias_sb[:])
            nc.sync.dma_start(
                out[tz + tt * P:tz + (tt + 1) * P, :],
                osbz[:, tt * ES:(tt + 1) * ES],
            )

        # last pair: transposes NOT deferred, so the per-t-tile staging DMAs
        # fire as early as possible
        g31 = emit_headpair(3, 1, at31, hook_tt=hook31, defer_tp=False,
                            ahead=2)
        drive(g30, v3, opr1[2:], 18, vstart=6, pre_gen=g31,
              pre_at=(11, 13, 15, 17))
        emit_ag(3, 0, at30)
        drive(g31, [], opr2, 14,
              at={14: [lambda: emit_tz_tt(0)], 16: [lambda: emit_tz_tt(1)]})
        drain(opr2)
        emit_tz_tt(2)
        emit_tz_tt(3)

    nc.compile()
    return nc


_NC_CACHE = {}


def _get_nc(with_collective=True):
    key = with_collective
    if key not in _NC_CACHE:
        _NC_CACHE[key] = build_nc(with_collective)
    return _NC_CACHE[key]


def make_in_maps(x, Wq, Wk, Wv, Wo, bo):
    import ml_dtypes
    bf16 = ml_dtypes.bfloat16
    f8 = ml_dtypes.float8_e4m3
    trbo = np.concatenate(
        [np.triu(np.ones((P, P), dtype=np.float32)),
         np.eye(P, dtype=np.float32),
         np.zeros((P, ES), dtype=np.float32)], axis=1)
    in_maps = []
    for c in range(N_CORES):
        b, g = c // GROUPS, c % GROUPS
        hs_ = slice(g * HPG, (g + 1) * HPG)
        tb = trbo.copy()
        tb[:, 2 * P:] = bo[g * ES:(g + 1) * ES][None, :]

        def shuffle8(W):
            # [C, HD] -> SBUF image [P, NCT*HD] (chunk ci at cols ci*HD)
            w = (W[hs_].transpose(1, 0, 2).reshape(C, HD) * WSC)
            return np.ascontiguousarray(
                w.reshape(NCT, P, HD).transpose(1, 0, 2).reshape(
                    P, NCT * HD)).astype(f8)

        in_maps.append({
            "xT": np.ascontiguousarray(x[b].T).astype(bf16),
            "x8": np.ascontiguousarray(x[b].T * XSC).astype(f8),
            "wq": shuffle8(Wq),
            "wk": shuffle8(Wk),
            "wv": np.ascontiguousarray(
                Wv[hs_].transpose(1, 0, 2).reshape(C, HD)).astype(bf16),
            "wo": np.ascontiguousarray(
                Wo[:, g * ES:(g + 1) * ES]).astype(bf16),
            "trbo": np.ascontiguousarray(tb).astype(bf16),
        })
    return in_maps


def kernel(x, Wq, Wk, Wv, Wo, bo):
    x = np.asarray(x, dtype=np.float32)
    Wq = np.asarray(Wq, dtype=np.float32)
    Wk = np.asarray(Wk, dtype=np.float32)
    Wv = np.asarray(Wv, dtype=np.float32)
    Wo = np.asarray(Wo, dtype=np.float32)
    bo = np.asarray(bo, dtype=np.float32)

    nc = _get_nc(with_collective=True)
    in_maps = make_in_maps(x, Wq, Wk, Wv, Wo, bo)
    res = run_bass_kernel_spmd(nc, in_maps, core_ids=list(range(N_CORES)))

    out = np.empty((B, T, E), dtype=np.float32)
    for c in range(N_CORES):
        b, g = c // GROUPS, c % GROUPS
        out[b, :, g * ES:(g + 1) * ES] = np.asarray(
            res.results[c]["out"], dtype=np.float32)
    return out


# revision 30
# speedup vs baseline: 1.0153x; 1.0114x over previous
"""Multi-head causal self-attention on 8 TRN2 NeuronCores (bf16 + fp8 QK).

Problem (nn_MultiHeadAttention): B=2, T=2048, C=1024, H=16 heads, hs=64.
  q,k,v = per-head projections of x; causal softmax(q k^T / 8) v;
  concat heads; out = att @ Wo + bo.

Sharding: core c in 0..7 -> (batch b = c//4, head-group g = c%4, 4 heads each).
Each core computes Q/K/V + causal attention for its 4 heads on its batch,
normalized attention outputs are AllGathered across the 4 cores of the same
batch (replica groups [0-3], [4-7]), then each core computes a disjoint
256-column slice of the output projection (column-parallel Wo) + bias slice.
Host does a pure concat of the 8 disjoint output slices.

Q/K projections run in fp8e4 DoubleRow mode: W (x64) and x (x4) are quantized
host-side, the PE contracts TWO 128-row c-chunks per pass at 0.5 cycles/row
(4x the bf16 rate), and the combined 2^16 scale is folded into the softmax
exp scale (0.125 / 65536).  Quantization error enters only through the
softmax logits, where it attenuates to ~1% of the output.  V, the attention
matmuls, and the output projection stay bf16 (their error hits the output
directly); PSUM accumulation is fp32 throughout.

Attention layout: scores stay transposed (S^T [s, t], lhsT=K^T, rhs=Q^T),
exp runs on ACT into bf16 P^T tiles, and AV is computed non-transposed:
out[t_p, d_f] += P^T[s, t-tile].T @ V[s, d] with full 128 output partitions.
Softmax denominators accumulate via F=1 ones-column matmuls into a shared
PSUM tile; normalization is a per-partition reciprocal + tensor_scalar
multiply on DVE.  The normalized A [t, d] tiles are transposed back to
A^T [d, t] on the PE for the AllGather + output projection.

Scheduling: the kernel is paced by ACT (exp) once PE work is cut by fp8, so
block 0 runs "scores-first": both head pairs' scores/exp for all four
s-tiles are emitted before any V-projection or AV work, keeping ACT fed
from ~3us while the x/W bf16 DMAs land.  Later blocks use the woven s-loop
schedule (scores si+1 emitted before AV si; per-(head,t-tile) AV bursts at
the diagonal step; projections/output-projections as pipeline fillers, one
block earlier than their consumers).
"""

import numpy as np
from contextlib import ExitStack

import concourse.bass as bass
import concourse.mybir as mybir
import concourse.tile as tile
from concourse import bacc
from concourse.bass_utils import run_bass_kernel_spmd

F32 = mybir.dt.float32
BF16 = mybir.dt.bfloat16
FP8 = mybir.dt.float8e4
EXP = mybir.ActivationFunctionType.Exp
DR = mybir.MatmulPerfMode.DoubleRow

N_CORES = 8
B = 2
T = 2048
C = 1024
NH = 16
HS = 64
E = 1024
GROUPS = 4          # head groups (tensor-parallel ranks per batch)
HPG = NH // GROUPS  # 4 heads per core
ES = E // GROUPS    # 256 output columns per core
HD = HPG * HS       # 256 local attention-output rows

P = 128             # partition tile
TBLK = 512          # t-block (matmul moving dim)
NTB = T // TBLK     # 4
NCT = C // P        # 8 contraction tiles for projections
NST = T // P        # 16 key tiles

WSC = 64.0          # host-side fp8 scale on Wq/Wk
XSC = 4.0           # host-side fp8 scale on x
EXPSC = 0.125 / (WSC * XSC) ** 2   # exp(EXPSC * S_scaled) = exp(S / sqrt(hs))

REPLICA_GROUPS = [[0, 1, 2, 3], [4, 5, 6, 7]]


def build_nc(with_collective=True):
    """Build + compile the per-core SPMD program. Same program on all cores."""
    nc = bacc.Bacc(
        "TRN2", target_bir_lowering=False, debug=False, num_devices=N_CORES
    )

    xT = nc.dram_tensor("xT", [C, T], BF16, kind="ExternalInput").ap()
    x8T = nc.dram_tensor("x8", [C, T], FP8, kind="ExternalInput").ap()
    # fp8 weights arrive pre-shuffled to the SBUF image [p, (c d)] so the DMA
    # moves one contiguous 2KB run per partition
    wq8 = nc.dram_tensor("wq", [P, NCT * ES], FP8, kind="ExternalInput").ap()
    wk8 = nc.dram_tensor("wk", [P, NCT * ES], FP8, kind="ExternalInput").ap()
    wv = nc.dram_tensor("wv", [C, HD], BF16, kind="ExternalInput").ap()
    wo = nc.dram_tensor("wo", [E, ES], BF16, kind="ExternalInput").ap()
    # tri (cols 0:P), identity (cols P:2P), broadcast bias (cols 2P:2P+ES)
    trbo = nc.dram_tensor("trbo", [P, 2 * P + ES], BF16,
                          kind="ExternalInput").ap()
    out = nc.dram_tensor("out", [T, ES], BF16, kind="ExternalOutput").ap()

    with tile.TileContext(nc) as tc, ExitStack() as ctx:
        wp = ctx.enter_context(tc.tile_pool(name="wp", bufs=1))
        xp = ctx.enter_context(tc.tile_pool(name="xp", bufs=3))
        x8p = ctx.enter_context(tc.tile_pool(name="x8p", bufs=3))
        qkp = ctx.enter_context(tc.tile_pool(name="qkp", bufs=1))
        vp = ctx.enter_context(tc.tile_pool(name="vp", bufs=1))
        ptp = ctx.enter_context(tc.tile_pool(name="ptp", bufs=22))
        anp = ctx.enter_context(tc.tile_pool(name="anp", bufs=4))
        atp = ctx.enter_context(tc.tile_pool(name="atp", bufs=3))
        smp = ctx.enter_context(tc.tile_pool(name="smp", bufs=6))
        outp = ctx.enter_context(tc.tile_pool(name="outp", bufs=4))
        lhp = ctx.enter_context(tc.tile_pool(name="lhp", bufs=10))
        # PSUM: 8 banks.  st [128,1024] x2 bufs = 4 banks, attps 1, denps 1,
        # small (qkv-proj / oproj / transpose) 2.
        ps2 = ctx.enter_context(tc.tile_pool(name="ps2", bufs=2, space="PSUM"))
        psA = ctx.enter_context(tc.tile_pool(name="psA", bufs=1, space="PSUM"))
        psD = ctx.enter_context(tc.tile_pool(name="psD", bufs=1, space="PSUM"))
        psC = ctx.enter_context(tc.tile_pool(name="psC", bufs=2, space="PSUM"))
        dramp = ctx.enter_context(tc.tile_pool(name="dramp", bufs=1, space="DRAM"))

        # ---- small constants ----
        trbo_sb = wp.tile([P, 2 * P + ES], BF16, tag="trbo")
        tri_sb = trbo_sb[:, 0:P]
        id_sb = trbo_sb[:, P:2 * P]
        bias_sb = trbo_sb[:, 2 * P:2 * P + ES]
        onescol = wp.tile([P, 1], BF16, tag="onescol")
        nc.vector.memset(onescol[:], 1.0)

        # weights: one wide tile per tensor, chunk ci at cols [ci*ES, ...)
        w_sb = {
            "wq": wp.tile([P, NCT * ES], FP8, tag="w_wq", name="w_wq"),
            "wk": wp.tile([P, NCT * ES], FP8, tag="w_wk", name="w_wk"),
            "wv": wp.tile([P, NCT * ES], BF16, tag="w_wv", name="w_wv"),
            "wo": wp.tile([P, NCT * ES], BF16, tag="w_wo", name="w_wo"),
        }

        def wsl(name, ci):
            return w_sb[name][:, ci * ES:(ci + 1) * ES]

        def w8sl(name, i, pr):
            # chunk-pair i of the fp8 weights as a DoubleRow lhsT
            # [p, 2 k-tiles, 128 out-rows] for head pair pr
            return w_sb[name][:, 2 * i * ES:(2 * i + 2) * ES].rearrange(
                "p (c m) -> p c m", m=ES)[:, :, pr * P:(pr + 1) * P]

        def emit_w_dma(name, dram, ci0, nch, eng=None):
            (eng or nc.sync).dma_start(
                w_sb[name][:, ci0 * ES:(ci0 + nch) * ES].rearrange(
                    "p (c d) -> p c d", d=ES),
                dram.rearrange("(c p) d -> p c d", p=P)[:, ci0:ci0 + nch, :],
            )

        def emit_w8_dma(name, dram, c0, c1, eng=None):
            (eng or nc.sync).dma_start(w_sb[name][:, c0:c1], dram[:, c0:c1])

        # denominators: slice (pr, head-in-pair, tt) -> one fp32 column
        denps = psD.tile([P, 16], F32, tag="denps")

        # x^T per t-block (bf16 for V): chunk ci at cols [ci*TBLK, ...)
        xw_of = {}

        def xsl(tb, ci):
            return xw_of[tb][:, ci * TBLK:(ci + 1) * TBLK]

        # x fp8 per t-block (for Q/K): chunk-pair i as DoubleRow rhs
        x8_of = {}

        def x8sl(tb, i):
            return x8_of[tb][:, 2 * i * TBLK:(2 * i + 2) * TBLK].rearrange(
                "p (c t) -> p c t", t=TBLK)

        # merged Q^T/K^T per head pair: col = tb*1024 + qk*512 + t_local
        # (pair p holds heads 2p (rows 0-63) and 2p+1 (rows 64-127))
        qkt = [qkp.tile([P, 2 * T], BF16, tag=f"qk{p_}", name=f"qk{p_}")
               for p_ in range(2)]

        def qt_slice(pr, r0, rn, t0, tn):
            tb, tl = t0 // TBLK, t0 % TBLK
            base = tb * 1024 + tl
            return qkt[pr][r0:r0 + rn, base:base + tn]

        def kt_slice(pr, r0, rn, s0, sn):
            tb, sl = s0 // TBLK, s0 % TBLK
            base = tb * 1024 + TBLK + sl
            return qkt[pr][r0:r0 + rn, base:base + sn]

        v_sb = [vp.tile([P, HPG * HS], BF16, tag=f"v{st}", name=f"v{st}")
                for st in range(NST)]

        # ---------------- stage-1 pieces ----------------
        def emit_x_dma(tb, eng=None):
            ts_ = tb * TBLK
            xw_of[tb] = xp.tile([P, NCT * TBLK], BF16, tag="xw",
                                name=f"xw{tb}")
            (eng or nc.sync).dma_start(
                xw_of[tb][:].rearrange("p (c t) -> p c t", t=TBLK),
                xT.rearrange("(c p) t -> p c t", p=P)[:, :, ts_:ts_ + TBLK],
            )

        def emit_x8_dma(tb, ci0=0, nch=NCT, eng=None):
            ts_ = tb * TBLK
            if tb not in x8_of:
                x8_of[tb] = x8p.tile([P, NCT * TBLK], FP8, tag="x8",
                                     name=f"x8_{tb}")
            (eng or nc.sync).dma_start(
                x8_of[tb][:, ci0 * TBLK:(ci0 + nch) * TBLK].rearrange(
                    "p (c t) -> p c t", t=TBLK),
                x8T.rearrange("(c p) t -> p c t", p=P)[
                    :, ci0:ci0 + nch, ts_:ts_ + TBLK],
            )

        def emit_qk_proj(tb, pr, which):
            # fp8 DoubleRow: 4 chunk-pair matmuls cover all 8 c-chunks at
            # 0.5 cycles/row -> ~0.43us of PE per (tb, pr, which)
            wn = "wq" if which == 0 else "wk"
            ps = psC.tile([P, TBLK], F32, tag="small",
                          name=f"qkps{tb}_{pr}_{which}")
            for i in range(4):
                nc.tensor.matmul(
                    ps[:], lhsT=w8sl(wn, i, pr), rhs=x8sl(tb, i),
                    start=(i == 0), stop=(i == 3), perf_mode=DR,
                )
            base = tb * 1024 + which * TBLK
            nc.vector.tensor_copy(qkt[pr][:, base:base + TBLK], ps[:])

        def qk_units(tb):
            # 4 units per t-block: (pr0,Q), (pr0,K), (pr1,Q), (pr1,K)
            return [lambda pr=pr, w=w: emit_qk_proj(tb, pr, w)
                    for pr in range(2) for w in range(2)]

        def emit_v_proj(st):
            tb, sl = st // 4, (st % 4) * P
            vps = psC.tile([P, TBLK], F32, tag="small", name=f"vps{st}")
            for ci in range(NCT):
                nc.tensor.matmul(
                    vps[:, 0:HD],
                    lhsT=xsl(tb, ci)[:, sl:sl + P],
                    rhs=wsl("wv", ci),
                    start=(ci == 0), stop=(ci == NCT - 1),
                )
            nc.vector.tensor_copy(v_sb[st][:], vps[:, 0:HD])

        def v_chunks(tb):
            return [lambda st=st: emit_v_proj(st)
                    for st in range(4 * tb, 4 * tb + 4)]

        # ------- stage-2: one head PAIR of one t-block ----------------------
        def emit_headpair(qb, pr, at, hook_tt=None, defer_tp=True, ahead=1):
            """s-loop over key tiles; both heads of the pair per step.  AV is
            non-transposed: attps[t_p, (par,tt) 64-col slice] with per-slice
            fp32 denominator columns in denps.  Each (par,tt) slice finishes
            at its diagonal s-step -> finalize (normalize + transpose into
            `at`) is woven in right there.

            `ahead` scores/exp steps are emitted before the first AV burst
            (ahead=1 is the classic software pipeline: scores(si+1) before
            AV(si); block 0 uses ahead=4 to front-run every exp past the
            V-projection DMA wall).  One yield per scores step, then one
            yield per s-tile of the burst phase.  hook_tt, if given, is
            called after finalize(tt)."""
            t0 = qb * TBLK
            ns = 4 * (qb + 1)
            attps = psA.tile([P, 4 * P], F32, tag="attps",
                             name=f"attps{qb}_{pr}")

            def scores_exp(si):
                diag = si * P >= t0
                ka = si * P - t0 if diag else 0
                stp = ps2.tile([P, 2 * TBLK], F32, tag="st",
                               name=f"st{qb}_{pr}_{si}")
                for par in range(2):
                    r0 = par * HS
                    nc.tensor.matmul(
                        stp[:, par * TBLK + ka:(par + 1) * TBLK],
                        lhsT=kt_slice(pr, r0, HS, si * P, P),
                        rhs=qt_slice(pr, r0, HS, t0 + ka, TBLK - ka),
                        start=True, stop=True,
                    )
                pt = ptp.tile([P, 2 * TBLK], BF16, tag="pt",
                              name=f"pt{qb}_{pr}_{si}")
                if ka > 0:
                    s3 = stp[:].rearrange("p (r c) -> p r c", r=2)[:, :, ka:TBLK]
                    p3 = pt[:].rearrange("p (r c) -> p r c", r=2)[:, :, ka:TBLK]
                    nc.scalar.activation(p3, s3, EXP, scale=EXPSC)
                else:
                    nc.scalar.activation(pt[:], stp[:], EXP, scale=EXPSC)
                return pt

            pending = []  # deferred transpose closures (one s-step later)

            def emit_tp(tt, an):
                tp = psC.tile([P, TBLK], BF16, tag="small",
                              name=f"tp{qb}_{pr}_{tt}",
                              padded_shape=[P, 2 * TBLK])
                nc.tensor.transpose(tp[:, 0:P], an[:], id_sb[:])
                nc.vector.tensor_copy(at[:, tt * P:(tt + 1) * P],
                                      tp[:, 0:P])
                if hook_tt is not None:
                    hook_tt(tt)

            def burst(si, tt, par, lo=0, hi=None, close=True):
                # den group first: its F=1 matmuls finish almost instantly,
                # so the DVE reciprocal overlaps the AV burst
                h = 2 * pr + par
                sl = (par * 4 + tt) * HS
                dc = pr * 8 + par * 4 + tt
                hi_ = si if hi is None else hi
                for sj in range(lo, hi_ + 1):
                    lhs = pts[sj][
                        :, par * TBLK + tt * P:par * TBLK + (tt + 1) * P]
                    nc.tensor.matmul(
                        denps[:, dc:dc + 1],
                        lhsT=lhs, rhs=onescol[:],
                        start=(sj == 0), stop=(close and sj == hi_),
                    )
                for sj in range(lo, hi_ + 1):
                    lhs = pts[sj][
                        :, par * TBLK + tt * P:par * TBLK + (tt + 1) * P]
                    nc.tensor.matmul(
                        attps[:, sl:sl + HS],
                        lhsT=lhs,
                        rhs=v_sb[sj][:, h * HS:(h + 1) * HS],
                        start=(sj == 0), stop=(close and sj == hi_),
                    )

            def norm(tt, par, an):
                # normalize on DVE; par0's chain overlaps par1's PE burst
                sl = (par * 4 + tt) * HS
                dc = pr * 8 + par * 4 + tt
                rc = smp.tile([P, 1], F32, tag="rc")
                nc.vector.reciprocal(rc[:], denps[:, dc:dc + 1])
                nc.vector.tensor_scalar_mul(
                    an[:, par * HS:(par + 1) * HS],
                    attps[:, sl:sl + HS], rc[:])

            # PSUM accumulation groups must not interleave within a bank on
            # real hardware, so AV for each (head, t-tile, par) is ONE
            # contiguous matmul burst over all its s-tiles, emitted at that
            # t-tile's diagonal s-step; every pt tile of the pair stays
            # resident until its last burst.  attps and denps are separate
            # banks, so their per-(head,t-tile) groups may alternate.  The
            # two par bursts straddle a yield so a filler or the next scores
            # emission lands between them (keeps the exp cadence smooth).
            pts = []
            for i in range(ahead):
                pts.append(scores_exp(i))
                yield
            for si in range(ns):
                if si + ahead < ns:
                    pts.append(scores_exp(si + ahead))
                diag = si * P >= t0
                if diag:
                    tt = (si * P - t0) // P
                    for par in range(2):
                        c0 = par * TBLK + tt * P
                        nc.vector.tensor_mul(
                            pts[si][:, c0:c0 + P], pts[si][:, c0:c0 + P],
                            tri_sb[:])
                    burst(si, tt, 0)
                yield
                while pending:
                    pending.pop(0)()
                if not diag:
                    continue
                burst(si, tt, 1)
                an = anp.tile([P, P], BF16, tag="an",
                              name=f"an{qb}_{pr}_{tt}")
                for par in range(2):
                    norm(tt, par, an)
                if defer_tp:
                    pending.append(lambda tt=tt, an=an: emit_tp(tt, an))
                else:
                    emit_tp(tt, an)
                yield
            while pending:
                pending.pop(0)()

        # ---------------- stage-3: one t-tile of one t-block ---------------
        # lh_of[qb] = [lhw_pr0, lhw_pr1]: wide AllGathered A^T tiles, rank g
        # at cols [g*TBLK, (g+1)*TBLK).  hdt (wo row-chunk) = 2g + pr.
        osb_of = {}

        def lh_sl(qb, hdt, tt):
            return lh_of[qb][hdt % 2][:, (hdt // 2) * TBLK + tt * P:
                                      (hdt // 2) * TBLK + (tt + 1) * P]

        def emit_oproj_tt(qb, tt, half=None, holder=None):
            t0 = qb * TBLK
            if half == 1:
                op = holder["ps"]
            else:
                op = psC.tile([P, TBLK], F32, tag="small", name=f"op{qb}_{tt}")
                if holder is not None:
                    holder["ps"] = op
            # pr0 tiles (even hdt) first: they arrive one AllGather earlier
            order = [0, 2, 4, 6, 1, 3, 5, 7]
            sel = order if half is None else order[4 * half:4 * half + 4]
            for i, hdt in enumerate(sel):
                nc.tensor.matmul(
                    op[:, 0:ES],
                    lhsT=lh_sl(qb, hdt, tt),
                    rhs=wsl("wo", hdt),
                    start=(half != 1 and i == 0),
                    stop=((half is None or half == 1) and i == len(sel) - 1),
                )
            if half == 0:
                return
            if qb not in osb_of:
                osb_of[qb] = (outp.tile([P, 4 * ES], BF16, tag="osb",
                                        name=f"osb{qb}"), [])
            osb, done = osb_of[qb]
            nc.vector.tensor_add(
                osb[:, tt * ES:(tt + 1) * ES], op[:, 0:ES], bias_sb[:])
            done.append(tt)
            if len(done) == 4:
                nc.sync.dma_start(
                    out[t0:t0 + TBLK, :].rearrange("(s p) e -> p s e", p=P),
                    osb[:].rearrange("p (s e) -> p s e", e=ES),
                )

        # --------- per-pair AllGather (pr = head pair 0/1 of this core) -----
        def emit_ag_cols(key, at, c0, cn):
            """Gather at[:, c0:c0+cn] across the 4 ranks of this batch;
            returns the wide SBUF tile with rank g at cols [g*cn, (g+1)*cn)."""
            ag_out = dramp.tile([GROUPS * P, cn], BF16, tag=f"agout{key}",
                                name=f"agout{key}")
            if with_collective:
                ag_in = dramp.tile([P, cn], BF16, tag=f"agin{key}",
                                   name=f"agin{key}")
                nc.sync.dma_start(ag_in[:], at[:, c0:c0 + cn])
                nc.gpsimd.collective_compute(
                    "AllGather",
                    mybir.AluOpType.bypass,
                    replica_groups=REPLICA_GROUPS,
                    ins=[ag_in[:].opt()],
                    outs=[ag_out[:].opt()],
                )
            else:
                # timing/sim variant: model the collective's local footprint
                # (own-contribution write; peer slots arrive over D2D, which
                # doesn't occupy the local DMA engines)
                nc.sync.dma_start(ag_out[0:P, :], at[:, c0:c0 + cn])
            lhw = lhp.tile([P, GROUPS * cn], BF16, tag="lh",
                           name=f"lh{key}")
            nc.sync.dma_start(
                lhw[:].rearrange("p (g t) -> p g t", g=GROUPS),
                ag_out[:].rearrange("(g p) t -> p g t", g=GROUPS),
            )
            return lhw

        def emit_ag(qb, pr, at):
            lh_of[qb][pr] = emit_ag_cols(f"{qb}_{pr}", at, 0, TBLK)

        # ---------------- emission schedule ----------------
        # startup DMAs in bus-priority order across two HWDGE queues; the
        # first Q/K DoubleRow matmul needs only wq8/x8(0) chunk-pair 0
        emit_w8_dma("wq", wq8, 0, NCT * ES)         # sync
        emit_x8_dma(0, 0, 2, eng=nc.scalar)         # scalar
        emit_w8_dma("wk", wk8, 0, NCT * ES)         # sync
        emit_x8_dma(0, 2, 6, eng=nc.scalar)         # scalar
        emit_x8_dma(1, eng=nc.sync)
        emit_w_dma("wv", wv, 0, NCT, eng=nc.scalar)
        nc.sync.dma_start(trbo_sb[:], trbo[:])
        emit_x_dma(0, eng=nc.scalar)                # bf16, for V
        emit_x_dma(1, eng=nc.sync)
        emit_w_dma("wo", wo, 0, NCT, eng=nc.scalar)

        def drive(gen, vfill, fillers, ns, vstart=None, pre_gen=None,
                  pre_at=(), at=None):
            """Drive a head pair's s-loop.  V fillers pop at ctrs
            vstart+1.. (early-mid loop, after their xT DMA but before their
            diagonal-step deadlines); generic fillers are spread evenly
            (Bresenham) over the remaining slots; pre_gen (the next pair) is
            advanced one step at each ctr in pre_at so its scores/exp keep
            ACT fed across the pair transition; `at` maps specific ctrs to
            extra closures (used to pin late work after a hook has fired)."""
            ctr = 0
            nf = len(fillers)
            done = 0
            vpops = 0
            if vstart is None:
                vstart = ns - len(vfill)
            den = max(1, ns - 2 - len(vfill))
            for _ in gen:
                ctr += 1
                if pre_gen is not None and ctr in pre_at:
                    next(pre_gen)
                if at and ctr in at:
                    for fn in at[ctr]:
                        fn()
                if vfill and ctr > vstart:
                    vfill.pop(0)()
                    vpops += 1
                else:
                    target = -(-(ctr - vpops) * nf // den)  # ceil
                    while done < target and fillers:
                        fillers.pop(0)()
                        done += 1

        lh_of = {qb: [None, None] for qb in range(NTB)}

        def new_at(qb, pr):
            return atp.tile([P, TBLK], BF16, tag="at", name=f"at{qb}_{pr}")

        def oproj_fillers(qb):
            # half-units (~0.45us) pack into sub-us pipeline holes
            units = []
            for tt in range(4):
                holder = {}
                units.append(lambda tt=tt, q=qb, h=holder:
                             emit_oproj_tt(q, tt, 0, h))
                units.append(lambda tt=tt, q=qb, h=holder:
                             emit_oproj_tt(q, tt, 1, h))
            return units

        def drain(lst):
            while lst:
                lst.pop(0)()

        def drain_gen(g):
            for _ in g:
                pass

        # ---- t-block 0: scores-first ----
        # PE order: p0 Q/K proj, all 8 scores/exp of both pairs (p1 proj and
        # block-1 p0 proj woven), then V + AV bursts with block-1 scores
        # trickling in via pre_at.  ACT runs exp back to back from ~5us
        # while xT/wv stream in for the V projections.
        qk0 = qk_units(0)
        qk1 = qk_units(1)
        qk0[0]()            # p0 Q
        qk0[1]()            # p0 K
        at00 = new_at(0, 0)
        at01 = new_at(0, 1)
        at10 = new_at(1, 0)
        g00 = emit_headpair(0, 0, at00, ahead=4)
        g01 = emit_headpair(0, 1, at01, ahead=4)
        g10 = emit_headpair(1, 0, at10, ahead=4)
        next(g00)           # s00_0
        next(g00)           # s00_1
        qk0[2]()            # p1 Q
        next(g00)           # s00_2
        qk0[3]()            # p1 K
        next(g00)           # s00_3
        for _ in range(4):
            next(g01)       # s01_0..3
        qk1[0]()            # block-1 p0 Q
        qk1[1]()            # block-1 p0 K
        v1 = v_chunks(1)
        next(g10)           # s10_0
        next(g10)           # s10_1  (ahead of the xT/wv DMA wall)
        v0 = v_chunks(0)
        v0.pop(0)()         # V(0) ahead of the first diagonal burst
        drive(g00, v0, [], 8, vstart=1, pre_gen=g10, pre_at=(3, 5))
        emit_ag(0, 0, at00)
        emit_x8_dma(2)
        emit_x_dma(2)
        drive(g01, v1[1:4], qk1[2:], 8, vstart=5, pre_gen=g10,
              pre_at=(1, 3, 5, 7))
        emit_ag(0, 1, at01)
        del v1[1:4]

        # ---- t-block 1 ----
        qk2 = qk_units(2)
        at11 = new_at(1, 1)
        g11 = emit_headpair(1, 1, at11, ahead=2)
        v1.pop(0)()         # V(4) ahead of g10's first diagonal burst
        drive(g10, v1, qk2[:2], 8, vstart=1, pre_gen=g11,
              pre_at=(1, 3, 5, 7))
        emit_ag(1, 0, at10)
        emit_x8_dma(3)
        emit_x_dma(3)
        opr0 = oproj_fillers(0)
        at20 = new_at(2, 0)
        g20 = emit_headpair(2, 0, at20, ahead=2)
        drive(g11, [], qk2[2:] + opr0[:4], 10, pre_gen=g20,
              pre_at=(1, 3, 5, 7))
        emit_ag(1, 1, at11)

        # ---- t-block 2 ----
        v2 = v_chunks(2)
        opr1 = oproj_fillers(1)
        at21 = new_at(2, 1)
        g21 = emit_headpair(2, 1, at21, ahead=2)
        drive(g20, v2, opr0[4:6], 14, vstart=2,
              pre_gen=g21, pre_at=(3, 5, 7, 9))
        emit_ag(2, 0, at20)
        qk3 = qk_units(3)
        at30 = new_at(3, 0)
        g30 = emit_headpair(3, 0, at30, ahead=2)
        drive(g21, [], qk3 + opr0[6:] + opr1[:2], 14, pre_gen=g30,
              pre_at=(3, 5, 7, 9))
        emit_ag(2, 1, at21)

        # ---- t-block 3 ----
        v3 = v_chunks(3)
        opr2 = oproj_fillers(2)
        at31 = new_at(3, 1)
        lh31 = {}
        # tt0+tt1 gather as one half; tt2 and tt3 each gather alone so the
        # final output projection only ever waits on the 128-col tile that
        # actually finished last
        agw = [2 * P, None, P, P]
        ag31out = {tt: dramp.tile([GROUPS * P, agw[tt]], BF16,
                                  tag=f"agout31{tt}", name=f"agout31{tt}")
                   for tt in (0, 2, 3)}
        ag31in = {tt: dramp.tile([P, agw[tt]], BF16, tag=f"agin31{tt}",
                                 name=f"agin31{tt}")
                  for tt in (0, 2, 3)}

        def hook31(tt):
            # stage each finished t-tile into its collective input as soon
            # as it exists; gathers ride the idle scalar queue
            key = 0 if tt < 2 else tt
            col = (tt % 2) * P if tt < 2 else 0
            src = at31[:, tt * P:(tt + 1) * P]
            stage = ag31in[key] if with_collective else ag31out[key]
            nc.sync.dma_start(stage[0:P, col:col + P], src)
            if tt == 0:
                return
            if with_collective:
                nc.gpsimd.collective_compute(
                    "AllGather",
                    mybir.AluOpType.bypass,
                    replica_groups=REPLICA_GROUPS,
                    ins=[ag31in[key][:].opt()],
                    outs=[ag31out[key][:].opt()],
                )
            lhw = lhp.tile([P, GROUPS * agw[key]], BF16, tag="lh",
                           name=f"lh31_{key}")
            nc.sync.dma_start(
                lhw[:].rearrange("p (g t) -> p g t", g=GROUPS),
                ag31out[key][:].rearrange("(g p) t -> p g t", g=GROUPS),
            )
            lh31[key] = lhw

        # final-block out-projection: one complete 8-matmul chain per t-tile
        # (pr-0 hd-tiles from AG(3,0), pr-1 from the half-AGs; in lh31
        # halves rank g sits at cols [g*256,+256), tt%2 picks the 128-col
        # t-tile).  The op accumulators live in the ps2 (scores) pool,
        # which is free once the last scores are emitted -- this keeps the
        # tail off the small-psum pool and its transpose-DMA WAR chains.
        # tt0/tt1 are injected into g31's loop right after the first
        # half-AllGather fires (via drive's `at`); tt2/tt3 follow the loop.
        tz = (NTB - 1) * TBLK
        osbz = outp.tile([P, 4 * ES], BF16, tag="osb", name="osbz")
        opz = [None]

        def emit_tz_tt(tt):
            if opz[0] is None:
                opz[0] = ps2.tile([P, 2 * TBLK], F32, tag="st", name="opz")
            op = opz[0][:, tt * ES:(tt + 1) * ES]
            key = 0 if tt < 2 else tt
            half = lh31[key]
            w = 2 * P if tt < 2 else P
            col = (tt % 2) * P if tt < 2 else 0
            for i, hdt in enumerate((0, 2, 4, 6)):
                nc.tensor.matmul(
                    op, lhsT=lh_sl(3, hdt, tt), rhs=wsl("wo", hdt),
                    start=(i == 0), stop=(i == 3),
                )
            for j, hdt in enumerate((1, 3, 5, 7)):
                g_ = (hdt - 1) // 2
                nc.tensor.matmul(
                    op,
                    lhsT=half[:, g_ * w + col:g_ * w + col + P],
                    rhs=wsl("wo", hdt),
                    start=False, stop=(j == 3),
                )
            nc.vector.tensor_add(
                osbz[:, tt * ES:(tt + 1) * ES], op, bias_sb[:])
            nc.sync.dma_start(
                out[tz + tt * P:tz + (tt + 1) * P, :],
                osbz[:, tt * ES:(tt + 1) * ES],
            )

        # last pair: transposes NOT deferred, so the per-t-tile staging DMAs
        # fire as early as possible
        g31 = emit_headpair(3, 1, at31, hook_tt=hook31, defer_tp=False,
                            ahead=2)
        drive(g30, v3, opr1[2:], 18, vstart=6, pre_gen=g31,
              pre_at=(7, 9, 11, 13))
        emit_ag(3, 0, at30)
        drive(g31, [], opr2, 14,
              at={14: [lambda: emit_tz_tt(0)], 16: [lambda: emit_tz_tt(1)]})
        drain(opr2)
        emit_tz_tt(2)
        emit_tz_tt(3)

    nc.compile()
    return nc


_NC_CACHE = {}


def _get_nc(with_collective=True):
    key = with_collective
    if key not in _NC_CACHE:
        _NC_CACHE[key] = build_nc(with_collective)
    return _NC_CACHE[key]


def make_in_maps(x, Wq, Wk, Wv, Wo, bo):
    import ml_dtypes
    bf16 = ml_dtypes.bfloat16
    f8 = ml_dtypes.float8_e4m3
    trbo = np.concatenate(
        [np.triu(np.ones((P, P), dtype=np.float32)),
         np.eye(P, dtype=np.float32),
         np.zeros((P, ES), dtype=np.float32)], axis=1)
    in_maps = []
    for c in range(N_CORES):
        b, g = c // GROUPS, c % GROUPS
        hs_ = slice(g * HPG, (g + 1) * HPG)
        tb = trbo.copy()
        tb[:, 2 * P:] = bo[g * ES:(g + 1) * ES][None, :]

        def shuffle8(W):
            # [C, HD] -> SBUF image [P, NCT*HD] (chunk ci at cols ci*HD)
            w = (W[hs_].transpose(1, 0, 2).reshape(C, HD) * WSC)
            return np.ascontiguousarray(
                w.reshape(NCT, P, HD).transpose(1, 0, 2).reshape(
                    P, NCT * HD)).astype(f8)

        in_maps.append({
            "xT": np.ascontiguousarray(x[b].T).astype(bf16),
            "x8": np.ascontiguousarray(x[b].T * XSC).astype(f8),
            "wq": shuffle8(Wq),
            "wk": shuffle8(Wk),
            "wv": np.ascontiguousarray(
                Wv[hs_].transpose(1, 0, 2).reshape(C, HD)).astype(bf16),
            "wo": np.ascontiguousarray(
                Wo[:, g * ES:(g + 1) * ES]).astype(bf16),
            "trbo": np.ascontiguousarray(tb).astype(bf16),
        })
    return in_maps


def kernel(x, Wq, Wk, Wv, Wo, bo):
    x = np.asarray(x, dtype=np.float32)
    Wq = np.asarray(Wq, dtype=np.float32)
    Wk = np.asarray(Wk, dtype=np.float32)
    Wv = np.asarray(Wv, dtype=np.float32)
    Wo = np.asarray(Wo, dtype=np.float32)
    bo = np.asarray(bo, dtype=np.float32)

    nc = _get_nc(with_collective=True)
    in_maps = make_in_maps(x, Wq, Wk, Wv, Wo, bo)
    res = run_bass_kernel_spmd(nc, in_maps, core_ids=list(range(N_CORES)))

    out = np.empty((B, T, E), dtype=np.float32)
    for c in range(N_CORES):
        b, g = c // GROUPS, c % GROUPS
        out[b, :, g * ES:(g + 1) * ES] = np.asarray(
            res.results[c]["out"], dtype=np.float32)
    return out


# revision 31
# speedup vs baseline: 1.0160x; 1.0006x over previous
"""Multi-head causal self-attention on 8 TRN2 NeuronCores (bf16 + fp8 QK).

Problem (nn_MultiHeadAttention): B=2, T=2048, C=1024, H=16 heads, hs=64.
  q,k,v = per-head projections of x; causal softmax(q k^T / 8) v;
  concat heads; out = att @ Wo + bo.

Sharding: core c in 0..7 -> (batch b = c//4, head-group g = c%4, 4 heads each).
Each core computes Q/K/V + causal attention for its 4 heads on its batch,
normalized attention outputs are AllGathered across the 4 cores of the same
batch (replica groups [0-3], [4-7]), then each core computes a disjoint
256-column slice of the output projection (column-parallel Wo) + bias slice.
Host does a pure concat of the 8 disjoint output slices.

Q/K projections run in fp8e4 DoubleRow mode: W (x64) and x (x4) are quantized
host-side, the PE contracts TWO 128-row c-chunks per pass at 0.5 cycles/row
(4x the bf16 rate), and the combined 2^16 scale is folded into the softmax
exp scale (0.125 / 65536).  Quantization error enters only through the
softmax logits, where it attenuates to ~1% of the output.  V, the attention
matmuls, and the output projection stay bf16 (their error hits the output
directly); PSUM accumulation is fp32 throughout.

Attention layout: scores stay transposed (S^T [s, t], lhsT=K^T, rhs=Q^T),
exp runs on ACT into bf16 P^T tiles, and AV is computed non-transposed:
out[t_p, d_f] += P^T[s, t-tile].T @ V[s, d] with full 128 output partitions.
Softmax denominators accumulate via F=1 ones-column matmuls into a shared
PSUM tile; normalization is a per-partition reciprocal + tensor_scalar
multiply on DVE.  The normalized A [t, d] tiles are transposed back to
A^T [d, t] on the PE for the AllGather + output projection.

Scheduling: the kernel is paced by ACT (exp) once PE work is cut by fp8, so
block 0 runs "scores-first": both head pairs' scores/exp for all four
s-tiles are emitted before any V-projection or AV work, keeping ACT fed
from ~3us while the x/W bf16 DMAs land.  Later blocks use the woven s-loop
schedule (scores si+1 emitted before AV si; per-(head,t-tile) AV bursts at
the diagonal step; projections/output-projections as pipeline fillers, one
block earlier than their consumers).
"""

import numpy as np
from contextlib import ExitStack

import concourse.bass as bass
import concourse.mybir as mybir
import concourse.tile as tile
from concourse import bacc
from concourse.bass_utils import run_bass_kernel_spmd

F32 = mybir.dt.float32
BF16 = mybir.dt.bfloat16
FP8 = mybir.dt.float8e4
EXP = mybir.ActivationFunctionType.Exp
DR = mybir.MatmulPerfMode.DoubleRow

N_CORES = 8
B = 2
T = 2048
C = 1024
NH = 16
HS = 64
E = 1024
GROUPS = 4          # head groups (tensor-parallel ranks per batch)
HPG = NH // GROUPS  # 4 heads per core
ES = E // GROUPS    # 256 output columns per core
HD = HPG * HS       # 256 local attention-output rows

P = 128             # partition tile
TBLK = 512          # t-block (matmul moving dim)
NTB = T // TBLK     # 4
NCT = C // P        # 8 contraction tiles for projections
NST = T // P        # 16 key tiles

WSC = 64.0          # host-side fp8 scale on Wq/Wk
XSC = 4.0           # host-side fp8 scale on x
EXPSC = 0.125 / (WSC * XSC) ** 2   # exp(EXPSC * S_scaled) = exp(S / sqrt(hs))

REPLICA_GROUPS = [[0, 1, 2, 3], [4, 5, 6, 7]]


def build_nc(with_collective=True):
    """Build + compile the per-core SPMD program. Same program on all cores."""
    nc = bacc.Bacc(
        "TRN2", target_bir_lowering=False, debug=False, num_devices=N_CORES
    )

    xT = nc.dram_tensor("xT", [C, T], BF16, kind="ExternalInput").ap()
    x8T = nc.dram_tensor("x8", [C, T], FP8, kind="ExternalInput").ap()
    # fp8 weights arrive pre-shuffled to the SBUF image [p, (c d)] so the DMA
    # moves one contiguous 2KB run per partition
    wq8 = nc.dram_tensor("wq", [P, NCT * ES], FP8, kind="ExternalInput").ap()
    wk8 = nc.dram_tensor("wk", [P, NCT * ES], FP8, kind="ExternalInput").ap()
    wv = nc.dram_tensor("wv", [C, HD], BF16, kind="ExternalInput").ap()
    wo = nc.dram_tensor("wo", [E, ES], BF16, kind="ExternalInput").ap()
    # tri (cols 0:P), identity (cols P:2P), broadcast bias (cols 2P:2P+ES)
    trbo = nc.dram_tensor("trbo", [P, 2 * P + ES], BF16,
                          kind="ExternalInput").ap()
    out = nc.dram_tensor("out", [T, ES], BF16, kind="ExternalOutput").ap()

    with tile.TileContext(nc) as tc, ExitStack() as ctx:
        wp = ctx.enter_context(tc.tile_pool(name="wp", bufs=1))
        xp = ctx.enter_context(tc.tile_pool(name="xp", bufs=3))
        x8p = ctx.enter_context(tc.tile_pool(name="x8p", bufs=3))
        qkp = ctx.enter_context(tc.tile_pool(name="qkp", bufs=1))
        vp = ctx.enter_context(tc.tile_pool(name="vp", bufs=1))
        ptp = ctx.enter_context(tc.tile_pool(name="ptp", bufs=22))
        anp = ctx.enter_context(tc.tile_pool(name="anp", bufs=4))
        atp = ctx.enter_context(tc.tile_pool(name="atp", bufs=3))
        smp = ctx.enter_context(tc.tile_pool(name="smp", bufs=6))
        outp = ctx.enter_context(tc.tile_pool(name="outp", bufs=4))
        lhp = ctx.enter_context(tc.tile_pool(name="lhp", bufs=10))
        # PSUM: 8 banks.  st [128,1024] x2 bufs = 4 banks, attps 1, denps 1,
        # small (qkv-proj / oproj / transpose) 2.
        ps2 = ctx.enter_context(tc.tile_pool(name="ps2", bufs=2, space="PSUM"))
        psA = ctx.enter_context(tc.tile_pool(name="psA", bufs=1, space="PSUM"))
        psD = ctx.enter_context(tc.tile_pool(name="psD", bufs=1, space="PSUM"))
        psC = ctx.enter_context(tc.tile_pool(name="psC", bufs=2, space="PSUM"))
        dramp = ctx.enter_context(tc.tile_pool(name="dramp", bufs=1, space="DRAM"))

        # ---- small constants ----
        trbo_sb = wp.tile([P, 2 * P + ES], BF16, tag="trbo")
        tri_sb = trbo_sb[:, 0:P]
        id_sb = trbo_sb[:, P:2 * P]
        bias_sb = trbo_sb[:, 2 * P:2 * P + ES]
        onescol = wp.tile([P, 1], BF16, tag="onescol")
        nc.vector.memset(onescol[:], 1.0)

        # weights: one wide tile per tensor, chunk ci at cols [ci*ES, ...)
        w_sb = {
            "wq": wp.tile([P, NCT * ES], FP8, tag="w_wq", name="w_wq"),
            "wk": wp.tile([P, NCT * ES], FP8, tag="w_wk", name="w_wk"),
            "wv": wp.tile([P, NCT * ES], BF16, tag="w_wv", name="w_wv"),
            "wo": wp.tile([P, NCT * ES], BF16, tag="w_wo", name="w_wo"),
        }

        def wsl(name, ci):
            return w_sb[name][:, ci * ES:(ci + 1) * ES]

        def w8sl(name, i, pr):
            # chunk-pair i of the fp8 weights as a DoubleRow lhsT
            # [p, 2 k-tiles, 128 out-rows] for head pair pr
            return w_sb[name][:, 2 * i * ES:(2 * i + 2) * ES].rearrange(
                "p (c m) -> p c m", m=ES)[:, :, pr * P:(pr + 1) * P]

        def emit_w_dma(name, dram, ci0, nch, eng=None):
            (eng or nc.sync).dma_start(
                w_sb[name][:, ci0 * ES:(ci0 + nch) * ES].rearrange(
                    "p (c d) -> p c d", d=ES),
                dram.rearrange("(c p) d -> p c d", p=P)[:, ci0:ci0 + nch, :],
            )

        def emit_w8_dma(name, dram, c0, c1, eng=None):
            (eng or nc.sync).dma_start(w_sb[name][:, c0:c1], dram[:, c0:c1])

        # denominators: slice (pr, head-in-pair, tt) -> one fp32 column
        denps = psD.tile([P, 16], F32, tag="denps")

        # x^T per t-block (bf16 for V): chunk ci at cols [ci*TBLK, ...)
        xw_of = {}

        def xsl(tb, ci):
            return xw_of[tb][:, ci * TBLK:(ci + 1) * TBLK]

        # x fp8 per t-block (for Q/K): chunk-pair i as DoubleRow rhs
        x8_of = {}

        def x8sl(tb, i):
            return x8_of[tb][:, 2 * i * TBLK:(2 * i + 2) * TBLK].rearrange(
                "p (c t) -> p c t", t=TBLK)

        # merged Q^T/K^T per head pair: col = tb*1024 + qk*512 + t_local
        # (pair p holds heads 2p (rows 0-63) and 2p+1 (rows 64-127))
        qkt = [qkp.tile([P, 2 * T], BF16, tag=f"qk{p_}", name=f"qk{p_}")
               for p_ in range(2)]

        def qt_slice(pr, r0, rn, t0, tn):
            tb, tl = t0 // TBLK, t0 % TBLK
            base = tb * 1024 + tl
            return qkt[pr][r0:r0 + rn, base:base + tn]

        def kt_slice(pr, r0, rn, s0, sn):
            tb, sl = s0 // TBLK, s0 % TBLK
            base = tb * 1024 + TBLK + sl
            return qkt[pr][r0:r0 + rn, base:base + sn]

        v_sb = [vp.tile([P, HPG * HS], BF16, tag=f"v{st}", name=f"v{st}")
                for st in range(NST)]

        # ---------------- stage-1 pieces ----------------
        def emit_x_dma(tb, eng=None):
            ts_ = tb * TBLK
            xw_of[tb] = xp.tile([P, NCT * TBLK], BF16, tag="xw",
                                name=f"xw{tb}")
            (eng or nc.sync).dma_start(
                xw_of[tb][:].rearrange("p (c t) -> p c t", t=TBLK),
                xT.rearrange("(c p) t -> p c t", p=P)[:, :, ts_:ts_ + TBLK],
            )

        def emit_x8_dma(tb, ci0=0, nch=NCT, eng=None):
            ts_ = tb * TBLK
            if tb not in x8_of:
                x8_of[tb] = x8p.tile([P, NCT * TBLK], FP8, tag="x8",
                                     name=f"x8_{tb}")
            (eng or nc.sync).dma_start(
                x8_of[tb][:, ci0 * TBLK:(ci0 + nch) * TBLK].rearrange(
                    "p (c t) -> p c t", t=TBLK),
                x8T.rearrange("(c p) t -> p c t", p=P)[
                    :, ci0:ci0 + nch, ts_:ts_ + TBLK],
            )

        def emit_qk_proj(tb, pr, which):
            # fp8 DoubleRow: 4 chunk-pair matmuls cover all 8 c-chunks at
            # 0.5 cycles/row -> ~0.43us of PE per (tb, pr, which)
            wn = "wq" if which == 0 else "wk"
            ps = psC.tile([P, TBLK], F32, tag="small",
                          name=f"qkps{tb}_{pr}_{which}")
            for i in range(4):
                nc.tensor.matmul(
                    ps[:], lhsT=w8sl(wn, i, pr), rhs=x8sl(tb, i),
                    start=(i == 0), stop=(i == 3), perf_mode=DR,
                )
            base = tb * 1024 + which * TBLK
            nc.vector.tensor_copy(qkt[pr][:, base:base + TBLK], ps[:])

        def qk_units(tb):
            # 4 units per t-block: (pr0,Q), (pr0,K), (pr1,Q), (pr1,K)
            return [lambda pr=pr, w=w: emit_qk_proj(tb, pr, w)
                    for pr in range(2) for w in range(2)]

        def emit_v_proj(st):
            tb, sl = st // 4, (st % 4) * P
            vps = psC.tile([P, TBLK], F32, tag="small", name=f"vps{st}")
            for ci in range(NCT):
                nc.tensor.matmul(
                    vps[:, 0:HD],
                    lhsT=xsl(tb, ci)[:, sl:sl + P],
                    rhs=wsl("wv", ci),
                    start=(ci == 0), stop=(ci == NCT - 1),
                )
            nc.vector.tensor_copy(v_sb[st][:], vps[:, 0:HD])

        def v_chunks(tb):
            return [lambda st=st: emit_v_proj(st)
                    for st in range(4 * tb, 4 * tb + 4)]

        # ------- stage-2: one head PAIR of one t-block ----------------------
        def emit_headpair(qb, pr, at, hook_tt=None, defer_tp=True, ahead=1):
            """s-loop over key tiles; both heads of the pair per step.  AV is
            non-transposed: attps[t_p, (par,tt) 64-col slice] with per-slice
            fp32 denominator columns in denps.  Each (par,tt) slice finishes
            at its diagonal s-step -> finalize (normalize + transpose into
            `at`) is woven in right there.

            `ahead` scores/exp steps are emitted before the first AV burst
            (ahead=1 is the classic software pipeline: scores(si+1) before
            AV(si); block 0 uses ahead=4 to front-run every exp past the
            V-projection DMA wall).  One yield per scores step, then one
            yield per s-tile of the burst phase.  hook_tt, if given, is
            called after finalize(tt)."""
            t0 = qb * TBLK
            ns = 4 * (qb + 1)
            attps = psA.tile([P, 4 * P], F32, tag="attps",
                             name=f"attps{qb}_{pr}")

            def scores_exp(si):
                diag = si * P >= t0
                ka = si * P - t0 if diag else 0
                stp = ps2.tile([P, 2 * TBLK], F32, tag="st",
                               name=f"st{qb}_{pr}_{si}")
                for par in range(2):
                    r0 = par * HS
                    nc.tensor.matmul(
                        stp[:, par * TBLK + ka:(par + 1) * TBLK],
                        lhsT=kt_slice(pr, r0, HS, si * P, P),
                        rhs=qt_slice(pr, r0, HS, t0 + ka, TBLK - ka),
                        start=True, stop=True,
                    )
                pt = ptp.tile([P, 2 * TBLK], BF16, tag="pt",
                              name=f"pt{qb}_{pr}_{si}")
                if ka > 0:
                    s3 = stp[:].rearrange("p (r c) -> p r c", r=2)[:, :, ka:TBLK]
                    p3 = pt[:].rearrange("p (r c) -> p r c", r=2)[:, :, ka:TBLK]
                    nc.scalar.activation(p3, s3, EXP, scale=EXPSC)
                else:
                    nc.scalar.activation(pt[:], stp[:], EXP, scale=EXPSC)
                return pt

            pending = []  # deferred transpose closures (one s-step later)

            def emit_tp(tt, an):
                tp = psC.tile([P, TBLK], BF16, tag="small",
                              name=f"tp{qb}_{pr}_{tt}",
                              padded_shape=[P, 2 * TBLK])
                nc.tensor.transpose(tp[:, 0:P], an[:], id_sb[:])
                nc.vector.tensor_copy(at[:, tt * P:(tt + 1) * P],
                                      tp[:, 0:P])
                if hook_tt is not None:
                    hook_tt(tt)

            def burst(si, tt, par, lo=0, hi=None, close=True):
                # den group first: its F=1 matmuls finish almost instantly,
                # so the DVE reciprocal overlaps the AV burst
                h = 2 * pr + par
                sl = (par * 4 + tt) * HS
                dc = pr * 8 + par * 4 + tt
                hi_ = si if hi is None else hi
                for sj in range(lo, hi_ + 1):
                    lhs = pts[sj][
                        :, par * TBLK + tt * P:par * TBLK + (tt + 1) * P]
                    nc.tensor.matmul(
                        denps[:, dc:dc + 1],
                        lhsT=lhs, rhs=onescol[:],
                        start=(sj == 0), stop=(close and sj == hi_),
                    )
                for sj in range(lo, hi_ + 1):
                    lhs = pts[sj][
                        :, par * TBLK + tt * P:par * TBLK + (tt + 1) * P]
                    nc.tensor.matmul(
                        attps[:, sl:sl + HS],
                        lhsT=lhs,
                        rhs=v_sb[sj][:, h * HS:(h + 1) * HS],
                        start=(sj == 0), stop=(close and sj == hi_),
                    )

            def norm(tt, par, an):
                # normalize on DVE; par0's chain overlaps par1's PE burst
                sl = (par * 4 + tt) * HS
                dc = pr * 8 + par * 4 + tt
                rc = smp.tile([P, 1], F32, tag="rc")
                nc.vector.reciprocal(rc[:], denps[:, dc:dc + 1])
                nc.vector.tensor_scalar_mul(
                    an[:, par * HS:(par + 1) * HS],
                    attps[:, sl:sl + HS], rc[:])

            # PSUM accumulation groups must not interleave within a bank on
            # real hardware, so AV for each (head, t-tile, par) is ONE
            # contiguous matmul burst over all its s-tiles, emitted at that
            # t-tile's diagonal s-step; every pt tile of the pair stays
            # resident until its last burst.  attps and denps are separate
            # banks, so their per-(head,t-tile) groups may alternate.  The
            # two par bursts straddle a yield so a filler or the next scores
            # emission lands between them (keeps the exp cadence smooth).
            pts = []
            for i in range(ahead):
                pts.append(scores_exp(i))
                yield
            for si in range(ns):
                if si + ahead < ns:
                    pts.append(scores_exp(si + ahead))
                diag = si * P >= t0
                if diag:
                    tt = (si * P - t0) // P
                    for par in range(2):
                        c0 = par * TBLK + tt * P
                        nc.vector.tensor_mul(
                            pts[si][:, c0:c0 + P], pts[si][:, c0:c0 + P],
                            tri_sb[:])
                    burst(si, tt, 0)
                yield
                while pending:
                    pending.pop(0)()
                if not diag:
                    continue
                burst(si, tt, 1)
                an = anp.tile([P, P], BF16, tag="an",
                              name=f"an{qb}_{pr}_{tt}")
                for par in range(2):
                    norm(tt, par, an)
                if defer_tp:
                    pending.append(lambda tt=tt, an=an: emit_tp(tt, an))
                else:
                    emit_tp(tt, an)
                yield
            while pending:
                pending.pop(0)()

        # ---------------- stage-3: one t-tile of one t-block ---------------
        # lh_of[qb] = [lhw_pr0, lhw_pr1]: wide AllGathered A^T tiles, rank g
        # at cols [g*TBLK, (g+1)*TBLK).  hdt (wo row-chunk) = 2g + pr.
        osb_of = {}

        def lh_sl(qb, hdt, tt):
            return lh_of[qb][hdt % 2][:, (hdt // 2) * TBLK + tt * P:
                                      (hdt // 2) * TBLK + (tt + 1) * P]

        def emit_oproj_tt(qb, tt, half=None, holder=None):
            t0 = qb * TBLK
            if half == 1:
                op = holder["ps"]
            else:
                op = psC.tile([P, TBLK], F32, tag="small", name=f"op{qb}_{tt}")
                if holder is not None:
                    holder["ps"] = op
            # pr0 tiles (even hdt) first: they arrive one AllGather earlier
            order = [0, 2, 4, 6, 1, 3, 5, 7]
            sel = order if half is None else order[4 * half:4 * half + 4]
            for i, hdt in enumerate(sel):
                nc.tensor.matmul(
                    op[:, 0:ES],
                    lhsT=lh_sl(qb, hdt, tt),
                    rhs=wsl("wo", hdt),
                    start=(half != 1 and i == 0),
                    stop=((half is None or half == 1) and i == len(sel) - 1),
                )
            if half == 0:
                return
            if qb not in osb_of:
                osb_of[qb] = (outp.tile([P, 4 * ES], BF16, tag="osb",
                                        name=f"osb{qb}"), [])
            osb, done = osb_of[qb]
            nc.vector.tensor_add(
                osb[:, tt * ES:(tt + 1) * ES], op[:, 0:ES], bias_sb[:])
            done.append(tt)
            if len(done) == 4:
                nc.sync.dma_start(
                    out[t0:t0 + TBLK, :].rearrange("(s p) e -> p s e", p=P),
                    osb[:].rearrange("p (s e) -> p s e", e=ES),
                )

        # --------- per-pair AllGather (pr = head pair 0/1 of this core) -----
        def emit_ag_cols(key, at, c0, cn):
            """Gather at[:, c0:c0+cn] across the 4 ranks of this batch;
            returns the wide SBUF tile with rank g at cols [g*cn, (g+1)*cn)."""
            ag_out = dramp.tile([GROUPS * P, cn], BF16, tag=f"agout{key}",
                                name=f"agout{key}")
            if with_collective:
                ag_in = dramp.tile([P, cn], BF16, tag=f"agin{key}",
                                   name=f"agin{key}")
                nc.sync.dma_start(ag_in[:], at[:, c0:c0 + cn])
                nc.gpsimd.collective_compute(
                    "AllGather",
                    mybir.AluOpType.bypass,
                    replica_groups=REPLICA_GROUPS,
                    ins=[ag_in[:].opt()],
                    outs=[ag_out[:].opt()],
                )
            else:
                # timing/sim variant: model the collective's local footprint
                # (own-contribution write; peer slots arrive over D2D, which
                # doesn't occupy the local DMA engines)
                nc.sync.dma_start(ag_out[0:P, :], at[:, c0:c0 + cn])
            lhw = lhp.tile([P, GROUPS * cn], BF16, tag="lh",
                           name=f"lh{key}")
            nc.sync.dma_start(
                lhw[:].rearrange("p (g t) -> p g t", g=GROUPS),
                ag_out[:].rearrange("(g p) t -> p g t", g=GROUPS),
            )
            return lhw

        def emit_ag(qb, pr, at):
            lh_of[qb][pr] = emit_ag_cols(f"{qb}_{pr}", at, 0, TBLK)

        # ---------------- emission schedule ----------------
        # startup DMAs in bus-priority order across two HWDGE queues; the
        # first Q/K DoubleRow matmul needs only wq8/x8(0) chunk-pair 0
        emit_w8_dma("wq", wq8, 0, NCT * ES)         # sync
        emit_x8_dma(0, 0, 2, eng=nc.scalar)         # scalar
        emit_w8_dma("wk", wk8, 0, NCT * ES)         # sync
        emit_x8_dma(0, 2, 6, eng=nc.scalar)         # scalar
        emit_x8_dma(1, eng=nc.sync)
        emit_w_dma("wv", wv, 0, NCT, eng=nc.scalar)
        nc.sync.dma_start(trbo_sb[:], trbo[:])
        emit_x_dma(0, eng=nc.scalar)                # bf16, for V
        emit_x_dma(1, eng=nc.sync)
        emit_w_dma("wo", wo, 0, NCT, eng=nc.scalar)

        def drive(gen, vfill, fillers, ns, vstart=None, pre_gen=None,
                  pre_at=(), at=None):
            """Drive a head pair's s-loop.  V fillers pop at ctrs
            vstart+1.. (early-mid loop, after their xT DMA but before their
            diagonal-step deadlines); generic fillers are spread evenly
            (Bresenham) over the remaining slots; pre_gen (the next pair) is
            advanced one step at each ctr in pre_at so its scores/exp keep
            ACT fed across the pair transition; `at` maps specific ctrs to
            extra closures (used to pin late work after a hook has fired)."""
            ctr = 0
            nf = len(fillers)
            done = 0
            vpops = 0
            if vstart is None:
                vstart = ns - len(vfill)
            den = max(1, ns - 2 - len(vfill))
            for _ in gen:
                ctr += 1
                if pre_gen is not None and ctr in pre_at:
                    next(pre_gen)
                if at and ctr in at:
                    for fn in at[ctr]:
                        fn()
                if vfill and ctr > vstart:
                    vfill.pop(0)()
                    vpops += 1
                else:
                    target = -(-(ctr - vpops) * nf // den)  # ceil
                    while done < target and fillers:
                        fillers.pop(0)()
                        done += 1

        lh_of = {qb: [None, None] for qb in range(NTB)}

        def new_at(qb, pr):
            return atp.tile([P, TBLK], BF16, tag="at", name=f"at{qb}_{pr}")

        def oproj_fillers(qb):
            # half-units (~0.45us) pack into sub-us pipeline holes
            units = []
            for tt in range(4):
                holder = {}
                units.append(lambda tt=tt, q=qb, h=holder:
                             emit_oproj_tt(q, tt, 0, h))
                units.append(lambda tt=tt, q=qb, h=holder:
                             emit_oproj_tt(q, tt, 1, h))
            return units

        def drain(lst):
            while lst:
                lst.pop(0)()

        def drain_gen(g):
            for _ in g:
                pass

        # ---- t-block 0: scores-first ----
        # PE order: p0 Q/K proj, all 8 scores/exp of both pairs (p1 proj and
        # block-1 p0 proj woven), then V + AV bursts with block-1 scores
        # trickling in via pre_at.  ACT runs exp back to back from ~5us
        # while xT/wv stream in for the V projections.
        qk0 = qk_units(0)
        qk1 = qk_units(1)
        qk0[0]()            # p0 Q
        qk0[1]()            # p0 K
        at00 = new_at(0, 0)
        at01 = new_at(0, 1)
        at10 = new_at(1, 0)
        g00 = emit_headpair(0, 0, at00, ahead=4)
        g01 = emit_headpair(0, 1, at01, ahead=4)
        g10 = emit_headpair(1, 0, at10, ahead=4)
        next(g00)           # s00_0
        next(g00)           # s00_1
        qk0[2]()            # p1 Q
        next(g00)           # s00_2
        qk0[3]()            # p1 K
        next(g00)           # s00_3
        for _ in range(4):
            next(g01)       # s01_0..3
        qk1[0]()            # block-1 p0 Q
        qk1[1]()            # block-1 p0 K
        v1 = v_chunks(1)
        next(g10)           # s10_0
        next(g10)           # s10_1  (ahead of the xT/wv DMA wall)
        v0 = v_chunks(0)
        v0.pop(0)()         # V(0) ahead of the first diagonal burst
        drive(g00, v0, [], 8, vstart=1, pre_gen=g10, pre_at=(3, 5))
        emit_ag(0, 0, at00)
        emit_x8_dma(2)
        emit_x_dma(2)
        drive(g01, v1[1:4], qk1[2:], 8, vstart=5, pre_gen=g10,
              pre_at=(1, 2, 3, 4))
        emit_ag(0, 1, at01)
        del v1[1:4]

        # ---- t-block 1 ----
        qk2 = qk_units(2)
        at11 = new_at(1, 1)
        g11 = emit_headpair(1, 1, at11, ahead=2)
        v1.pop(0)()         # V(4) ahead of g10's first diagonal burst
        drive(g10, v1, qk2[:2], 8, vstart=1, pre_gen=g11,
              pre_at=(1, 2, 3, 4))
        emit_ag(1, 0, at10)
        emit_x8_dma(3)
        emit_x_dma(3)
        opr0 = oproj_fillers(0)
        at20 = new_at(2, 0)
        g20 = emit_headpair(2, 0, at20, ahead=2)
        drive(g11, [], qk2[2:] + opr0[:4], 10, pre_gen=g20,
              pre_at=(1, 3, 5, 7))
        emit_ag(1, 1, at11)

        # ---- t-block 2 ----
        v2 = v_chunks(2)
        opr1 = oproj_fillers(1)
        at21 = new_at(2, 1)
        g21 = emit_headpair(2, 1, at21, ahead=2)
        drive(g20, v2, opr0[4:6], 14, vstart=2,
              pre_gen=g21, pre_at=(3, 5, 7, 9))
        emit_ag(2, 0, at20)
        qk3 = qk_units(3)
        at30 = new_at(3, 0)
        g30 = emit_headpair(3, 0, at30, ahead=2)
        drive(g21, [], qk3 + opr0[6:] + opr1[:2], 14, pre_gen=g30,
              pre_at=(3, 5, 7, 9))
        emit_ag(2, 1, at21)

        # ---- t-block 3 ----
        v3 = v_chunks(3)
        opr2 = oproj_fillers(2)
        at31 = new_at(3, 1)
        lh31 = {}
        # tt0+tt1 gather as one half; tt2 and tt3 each gather alone so the
        # final output projection only ever waits on the 128-col tile that
        # actually finished last
        agw = [2 * P, None, P, P]
        ag31out = {tt: dramp.tile([GROUPS * P, agw[tt]], BF16,
                                  tag=f"agout31{tt}", name=f"agout31{tt}")
                   for tt in (0, 2, 3)}
        ag31in = {tt: dramp.tile([P, agw[tt]], BF16, tag=f"agin31{tt}",
                                 name=f"agin31{tt}")
                  for tt in (0, 2, 3)}

        def hook31(tt):
            # stage each finished t-tile into its collective input as soon
            # as it exists; gathers ride the idle scalar queue
            key = 0 if tt < 2 else tt
            col = (tt % 2) * P if tt < 2 else 0
            src = at31[:, tt * P:(tt + 1) * P]
            stage = ag31in[key] if with_collective else ag31out[key]
            nc.sync.dma_start(stage[0:P, col:col + P], src)
            if tt == 0:
                return
            if with_collective:
                nc.gpsimd.collective_compute(
                    "AllGather",
                    mybir.AluOpType.bypass,
                    replica_groups=REPLICA_GROUPS,
                    ins=[ag31in[key][:].opt()],
                    outs=[ag31out[key][:].opt()],
                )
            lhw = lhp.tile([P, GROUPS * agw[key]], BF16, tag="lh",
                           name=f"lh31_{key}")
            nc.sync.dma_start(
                lhw[:].rearrange("p (g t) -> p g t", g=GROUPS),
                ag31out[key][:].rearrange("(g p) t -> p g t", g=GROUPS),
            )
            lh31[key] = lhw

        # final-block out-projection: one complete 8-matmul chain per t-tile
        # (pr-0 hd-tiles from AG(3,0), pr-1 from the half-AGs; in lh31
        # halves rank g sits at cols [g*256,+256), tt%2 picks the 128-col
        # t-tile).  The op accumulators live in the ps2 (scores) pool,
        # which is free once the last scores are emitted -- this keeps the
        # tail off the small-psum pool and its transpose-DMA WAR chains.
        # tt0/tt1 are injected into g31's loop right after the first
        # half-AllGather fires (via drive's `at`); tt2/tt3 follow the loop.
        tz = (NTB - 1) * TBLK
        osbz = outp.tile([P, 4 * ES], BF16, tag="osb", name="osbz")
        opz = [None]

        def emit_tz_tt(tt):
            if opz[0] is None:
                opz[0] = ps2.tile([P, 2 * TBLK], F32, tag="st", name="opz")
            op = opz[0][:, tt * ES:(tt + 1) * ES]
            key = 0 if tt < 2 else tt
            half = lh31[key]
            w = 2 * P if tt < 2 else P
            col = (tt % 2) * P if tt < 2 else 0
            for i, hdt in enumerate((0, 2, 4, 6)):
                nc.tensor.matmul(
                    op, lhsT=lh_sl(3, hdt, tt), rhs=wsl("wo", hdt),
                    start=(i == 0), stop=(i == 3),
                )
            for j, hdt in enumerate((1, 3, 5, 7)):
                g_ = (hdt - 1) // 2
                nc.tensor.matmul(
                    op,
                    lhsT=half[:, g_ * w + col:g_ * w + col + P],
                    rhs=wsl("wo", hdt),
                    start=False, stop=(j == 3),
                )
            nc.vector.tensor_add(
                osbz[:, tt * ES:(tt + 1) * ES], op, bias_sb[:])
            nc.sync.dma_start(
                out[tz + tt * P:tz + (tt + 1) * P, :],
                osbz[:, tt * ES:(tt + 1) * ES],
            )

        # last pair: transposes NOT deferred, so the per-t-tile staging DMAs
        # fire as early as possible
        g31 = emit_headpair(3, 1, at31, hook_tt=hook31, defer_tp=False,
                            ahead=2)
        drive(g30, v3, opr1[2:], 18, vstart=6, pre_gen=g31,
              pre_at=(7, 9, 11, 13))
        emit_ag(3, 0, at30)
        drive(g31, [], opr2, 14,
              at={14: [lambda: emit_tz_tt(0)], 16: [lambda: emit_tz_tt(1)]})
        drain(opr2)
        emit_tz_tt(2)
        emit_tz_tt(3)

    nc.compile()
    return nc


_NC_CACHE = {}


def _get_nc(with_collective=True):
    key = with_collective
    if key not in _NC_CACHE:
        _NC_CACHE[key] = build_nc(with_collective)
    return _NC_CACHE[key]


def make_in_maps(x, Wq, Wk, Wv, Wo, bo):
    import ml_dtypes
    bf16 = ml_dtypes.bfloat16
    f8 = ml_dtypes.float8_e4m3
    trbo = np.concatenate(
        [np.triu(np.ones((P, P), dtype=np.float32)),
         np.eye(P, dtype=np.float32),
         np.zeros((P, ES), dtype=np.float32)], axis=1)
    in_maps = []
    for c in range(N_CORES):
        b, g = c // GROUPS, c % GROUPS
        hs_ = slice(g * HPG, (g + 1) * HPG)
        tb = trbo.copy()
        tb[:, 2 * P:] = bo[g * ES:(g + 1) * ES][None, :]

        def shuffle8(W):
            # [C, HD] -> SBUF image [P, NCT*HD] (chunk ci at cols ci*HD)
            w = (W[hs_].transpose(1, 0, 2).reshape(C, HD) * WSC)
            return np.ascontiguousarray(
                w.reshape(NCT, P, HD).transpose(1, 0, 2).reshape(
                    P, NCT * HD)).astype(f8)

        in_maps.append({
            "xT": np.ascontiguousarray(x[b].T).astype(bf16),
            "x8": np.ascontiguousarray(x[b].T * XSC).astype(f8),
            "wq": shuffle8(Wq),
            "wk": shuffle8(Wk),
            "wv": np.ascontiguousarray(
                Wv[hs_].transpose(1, 0, 2).reshape(C, HD)).astype(bf16),
            "wo": np.ascontiguousarray(
                Wo[:, g * ES:(g + 1) * ES]).astype(bf16),
            "trbo": np.ascontiguousarray(tb).astype(bf16),
        })
    return in_maps


def kernel(x, Wq, Wk, Wv, Wo, bo):
    x = np.asarray(x, dtype=np.float32)
    Wq = np.asarray(Wq, dtype=np.float32)
    Wk = np.asarray(Wk, dtype=np.float32)
    Wv = np.asarray(Wv, dtype=np.float32)
    Wo = np.asarray(Wo, dtype=np.float32)
    bo = np.asarray(bo, dtype=np.float32)

    nc = _get_nc(with_collective=True)
    in_maps = make_in_maps(x, Wq, Wk, Wv, Wo, bo)
    res = run_bass_kernel_spmd(nc, in_maps, core_ids=list(range(N_CORES)))

    out = np.empty((B, T, E), dtype=np.float32)
    for c in range(N_CORES):
        b, g = c // GROUPS, c % GROUPS
        out[b, :, g * ES:(g + 1) * ES] = np.asarray(
            res.results[c]["out"], dtype=np.float32)
    return out
